# revision 1
# baseline (speedup 1.0000x reference)
"""KNN-softmax loss kernel for Trainium2, SPMD over 8 NeuronCores.

Problem: N=8192 points, D=128, 128 classes, K=16, alpha=1.
reference computes pairwise euclidean distances, a per-row (K+1)-th smallest
off-diagonal threshold, masked exp-sums below the threshold split by label
match, and reduces to 4 scalars (loss, accuracy, tp, tn).

Sharding: rows of the distance matrix are data-parallel across 8 cores
(1024 rows each); every core holds all N column embeddings (X^T), computes
its [1024, 8192] distance block in 128-row tiles streamed through PSUM/SBUF,
and reduces to 3 per-row stats (sum of top-16 exp(-d), same-label part of
that sum, same-label count). The tiny O(N) postlude (fallback pairs,
valid mask, final scalars) runs on host.

Key device algorithm per 128-row tile:
  d2 = -2*X_r@X^T + ||x_r||^2 + ||x_c||^2 via PE (f32r), diagonal pushed to
  +inf; dist = sqrt(d2) and e = exp(-dist) on the scalar engine;
  top-16/17th values of e per row via grouped DVE max8 + match_replace
  (threshold = midpoint of 16th/17th largest); same-label-masked e handled
  the same way after multiplying by a label-equality mask.

Columns are rotated per core by c*1024 so each core's diagonal block lands
at a core-independent position (the SPMD program is identical on all cores).
"""

import numpy as np

N, D, NCORES = 8192, 128, 8
ROWS = N // NCORES          # rows per core
NRT = ROWS // 128           # row-tiles per core
NCH = 8                     # column chunks per row
CHW = N // NCH              # chunk width (1024)
GRP = 512                   # stage-1 top-8 group width
BIG = 1.0e12                # diagonal d2 offset

_CACHE = {}


def _build_program():
    import concourse.mybir as mybir
    import concourse.tile as tile
    from concourse import bacc

    f32 = mybir.dt.float32
    f32r = mybir.dt.float32r
    AX = mybir.AxisListType
    OP = mybir.AluOpType
    AF = mybir.ActivationFunctionType

    nc = bacc.Bacc(
        "TRN2", target_bir_lowering=False, debug=False, num_devices=NCORES
    )

    # blob layout: [0:N) xT | [N:N+ROWS) lhsT | [N+ROWS : N+ROWS+128) eyebig
    #              | +NRT sqrow | +NRT trow
    BW = N + ROWS + 128 + 2 * NRT
    blob = nc.dram_tensor("blob", [D, BW], f32, kind="ExternalInput").ap()
    sqcolB_d = nc.dram_tensor("sqcolB", [128, N], f32, kind="ExternalInput").ap()
    tcolB_d = nc.dram_tensor("tcolB", [128, N], f32, kind="ExternalInput").ap()
    outd = nc.dram_tensor("out", [128, NRT * 3], f32, kind="ExternalOutput").ap()

    with tile.TileContext(nc) as tc:
        with (
            tc.tile_pool(name="persist", bufs=1) as pp,
            tc.tile_pool(name="stream", bufs=3) as sp,
            tc.tile_pool(name="cand", bufs=2) as cp,
            tc.tile_pool(name="small", bufs=4) as smp,
            tc.tile_pool(name="psum", bufs=3, space="PSUM") as psp,
        ):
            # one blob DMA; everything extracted with single DVE copies so
            # downstream consumers only ever wait on the DVE semaphore
            st = pp.tile([D, BW], f32, tag="stage")
            nc.sync.dma_start(out=st[:], in_=blob)
            xT_sb = pp.tile([D, N], f32r, tag="xT")
            nc.vector.tensor_copy(xT_sb[:], st[:, 0:N])
            lhsT_sb = pp.tile([D, ROWS], f32r, tag="lhsT")
            nc.vector.tensor_copy(lhsT_sb[:], st[:, N : N + ROWS])
            o = N + ROWS
            eyeb_sb = pp.tile([128, 128], f32, tag="eyeb")
            nc.vector.tensor_copy(eyeb_sb[:], st[:, o : o + 128])
            sqrow_sb = pp.tile([128, NRT], f32, tag="sqrow")
            nc.vector.tensor_copy(sqrow_sb[:], st[:, o + 128 : o + 128 + NRT])
            trow_sb = pp.tile([128, NRT], f32, tag="trow")
            nc.vector.tensor_copy(
                trow_sb[:], st[:, o + 128 + NRT : o + 128 + 2 * NRT]
            )
            sqcolB = pp.tile([128, N], f32, tag="sqcolB")
            nc.sync.dma_start(out=sqcolB[:], in_=sqcolB_d)
            tcolB = pp.tile([128, N], f32, tag="tcolB")
            nc.sync.dma_start(out=tcolB[:], in_=tcolB_d)

            out_sb = pp.tile([128, NRT * 3], f32, tag="out")

            ngrp = N // GRP  # stage-1 groups per row
            for rt in range(NRT):
                ce = cp.tile([128, ngrp * 8], f32, tag="ce")
                cpos = cp.tile([128, ngrp * 8], f32, tag="cpos")
                for ch in range(NCH):
                    ps = psp.tile([128, CHW], f32, tag="ps")
                    for h in range(CHW // 512):
                        c0 = ch * CHW + h * 512
                        nc.tensor.matmul(
                            ps[:, h * 512 : (h + 1) * 512],
                            lhsT_sb[:, rt * 128 : (rt + 1) * 128],
                            xT_sb[:, c0 : c0 + 512],
                            start=True,
                            stop=True,
                        )
                    if ch == 0:
                        # push this row-tile's diagonal block to +inf
                        dsl = ps[:, rt * 128 : (rt + 1) * 128]
                        nc.vector.tensor_add(dsl, dsl, eyeb_sb[:])

                    # d2 = (psum + ||x_r||^2) + ||x_c||^2, then in-place sqrt
                    dist = sp.tile([128, CHW], f32, tag="dist")
                    nc.vector.scalar_tensor_tensor(
                        dist[:],
                        ps[:],
                        sqrow_sb[:, rt : rt + 1],
                        sqcolB[:, ch * CHW : (ch + 1) * CHW],
                        op0=OP.add,
                        op1=OP.add,
                    )
                    nc.scalar.activation(dist[:], dist[:], AF.Sqrt)
                    e = sp.tile([128, CHW], f32, tag="e")
                    nc.scalar.activation(e[:], dist[:], AF.Exp, scale=-1.0)

                    sm = sp.tile([128, CHW], f32, tag="sm")
                    nc.gpsimd.tensor_scalar(
                        sm[:],
                        tcolB[:, ch * CHW : (ch + 1) * CHW],
                        trow_sb[:, rt : rt + 1],
                        None,
                        op0=OP.is_equal,
                    )
                    ep = sp.tile([128, CHW], f32, tag="ep")
                    nc.vector.tensor_mul(ep[:], e[:], sm[:])

                    for g in range(CHW // GRP):
                        gi = ch * (CHW // GRP) + g
                        nc.vector.max(
                            ce[:, gi * 8 : gi * 8 + 8],
                            e[:, g * GRP : (g + 1) * GRP],
                        )
                        nc.vector.max(
                            cpos[:, gi * 8 : gi * 8 + 8],
                            ep[:, g * GRP : (g + 1) * GRP],
                        )

                # stage 2: exact top-16 + 17th value from the candidates
                m16 = smp.tile([128, 16], f32, tag="m16")
                m3 = smp.tile([128, 8], f32, tag="m3")
                ce2 = smp.tile([128, ngrp * 8], f32, tag="ce2")
                ce3 = smp.tile([128, ngrp * 8], f32, tag="ce3")
                nc.vector.max(m16[:, 0:8], ce[:])
                nc.vector.match_replace(
                    out=ce2[:], in_to_replace=m16[:, 0:8], in_values=ce[:],
                    imm_value=0.0,
                )
                nc.vector.max(m16[:, 8:16], ce2[:])
                nc.vector.match_replace(
                    out=ce3[:], in_to_replace=m16[:, 8:16], in_values=ce2[:],
                    imm_value=0.0,
                )
                nc.vector.max(m3[:], ce3[:])

                # s_tot = sum of top-16 e values
                nc.vector.tensor_reduce(
                    out_sb[:, rt * 3 : rt * 3 + 1], m16[:], axis=AX.X, op=OP.add
                )
                # threshold strictly between 16th and 17th largest
                thr = smp.tile([128, 1], f32, tag="thr")
                nc.vector.tensor_add(thr[:], m16[:, 15:16], m3[:, 0:1])
                nc.vector.tensor_scalar_mul(thr[:], thr[:], 0.5)

                # top-16 of the same-label-masked e
                mp16 = smp.tile([128, 16], f32, tag="mp16")
                cp2 = smp.tile([128, ngrp * 8], f32, tag="cp2")
                nc.vector.max(mp16[:, 0:8], cpos[:])
                nc.vector.match_replace(
                    out=cp2[:], in_to_replace=mp16[:, 0:8], in_values=cpos[:],
                    imm_value=0.0,
                )
                nc.vector.max(mp16[:, 8:16], cp2[:])

                maskp = smp.tile([128, 16], f32, tag="maskp")
                nc.vector.tensor_scalar(
                    maskp[:], mp16[:], thr[:], None, op0=OP.is_gt
                )
                nc.vector.tensor_reduce(
                    out_sb[:, rt * 3 + 2 : rt * 3 + 3], maskp[:], axis=AX.X,
                    op=OP.add,
                )
                prod = smp.tile([128, 16], f32, tag="prod")
                nc.vector.tensor_mul(prod[:], mp16[:], maskp[:])
                nc.vector.tensor_reduce(
                    out_sb[:, rt * 3 + 1 : rt * 3 + 2], prod[:], axis=AX.X,
                    op=OP.add,
                )

            nc.sync.dma_start(out=outd, in_=out_sb[:])

    nc.compile()
    return nc


def _host_inputs(X, T):
    """Per-core input dicts. Core c's columns are rotated by c*ROWS."""
    sq = np.sum(X.astype(np.float32) * X.astype(np.float32), axis=1)
    Tf = T.astype(np.float32)
    eyeb = (BIG * np.eye(128)).astype(np.float32)
    in_maps = []
    for c in range(NCORES):
        rot = np.roll(np.arange(N), -c * ROWS)
        rows = slice(c * ROWS, (c + 1) * ROWS)
        blob = np.concatenate(
            [
                X[rot].T.astype(np.float32),
                (-2.0 * X[rows]).T.astype(np.float32),
                eyeb,
                (sq[rows] + 1e-3).reshape(NRT, 128).T,
                Tf[rows].reshape(NRT, 128).T,
            ],
            axis=1,
        )
        in_maps.append(
            {
                "blob": np.ascontiguousarray(blob),
                "sqcolB": np.ascontiguousarray(
                    np.broadcast_to(sq[rot][None, :], (128, N))
                ),
                "tcolB": np.ascontiguousarray(
                    np.broadcast_to(Tf[rot][None, :], (128, N))
                ),
            }
        )
    return in_maps


def _postlude(X, T, s_tot, s_pos, cnt_pos):
    """Host finish: fallback pairs, valid mask, final 4 scalars."""
    n = N
    Xf = X.astype(np.float64)
    sq = np.sum(X.astype(np.float32) * X.astype(np.float32), axis=1).astype(
        np.float64
    )

    cnt_pos = np.round(cnt_pos).astype(np.int64)
    count_neg = 16 - cnt_pos
    neg_logit = s_tot.astype(np.float64) - s_pos.astype(np.float64)
    neg_logit = np.maximum(neg_logit, 0.0)

    # first same-label off-diagonal index per row (order of original columns)
    first_pos = np.zeros(n, dtype=np.int64)
    order = np.argsort(T, kind="stable")
    # build per-label sorted index lists
    from collections import defaultdict

    by_label = defaultdict(list)
    for idx in order:
        by_label[int(T[idx])].append(int(idx))
    for i in range(n):
        lst = by_label[int(T[i])]
        if len(lst) >= 2:
            first_pos[i] = lst[1] if lst[0] == i else lst[0]
        else:
            first_pos[i] = 0  # no positives; row is invalid anyway

    # fallback distance computed exactly like the reference formula
    j = first_pos
    d2 = sq + sq[j] - 2.0 * np.einsum("ij,ij->i", Xf, Xf[j])
    fb_dist = np.sqrt(np.maximum(d2, 1e-12))
    fallback = np.exp(-fb_dist)

    counts = np.bincount(T.astype(np.int64), minlength=128)
    same_cnt = counts[T.astype(np.int64)] - 1
    valid = (same_cnt > 0) & ((n - 1 - same_cnt) > 0)

    pos_eff = np.where(cnt_pos == 0, fallback, s_pos.astype(np.float64))
    loss_i = -np.log(pos_eff / (pos_eff + neg_logit))
    loss = np.sum(np.where(valid, loss_i, 0.0)) / n

    count_pos_acc = np.where(cnt_pos == 0, 1, cnt_pos)
    accuracy = np.sum((valid & (count_pos_acc > count_neg)).astype(np.float64)) / n
    tp = np.sum(np.where(valid, cnt_pos, 0)) / n
    tn = np.sum(np.where(valid, count_neg, 0)) / n
    return (
        np.float32(loss),
        np.float32(accuracy),
        np.float32(tp),
        np.float32(tn),
    )


def kernel(inputs, targets):
    from concourse.bass_utils import run_bass_kernel_spmd

    X = np.asarray(inputs, dtype=np.float32)
    T = np.asarray(targets).astype(np.int64)

    if "nc" not in _CACHE:
        _CACHE["nc"] = _build_program()
    nc = _CACHE["nc"]

    in_maps = _host_inputs(X, T)
    res = run_bass_kernel_spmd(nc, in_maps, core_ids=list(range(NCORES)))

    s_tot = np.zeros(N, dtype=np.float64)
    s_pos = np.zeros(N, dtype=np.float64)
    cnt_pos = np.zeros(N, dtype=np.float64)
    for c in range(NCORES):
        out = res.results[c]["out"]  # [128, NRT*3]
        for rt in range(NRT):
            g = slice(c * ROWS + rt * 128, c * ROWS + (rt + 1) * 128)
            s_tot[g] = out[:, rt * 3]
            s_pos[g] = out[:, rt * 3 + 1]
            cnt_pos[g] = out[:, rt * 3 + 2]

    return _postlude(X, T, s_tot, s_pos, cnt_pos)



# revision 4
# speedup vs baseline: 4.8532x; 4.8532x over previous
"""KNN-softmax loss kernel for Trainium2, SPMD over 8 NeuronCores.

Problem: N=8192 points, D=128, 128 classes, K=16, alpha=1.
reference computes pairwise euclidean distances, a per-row 17th-smallest
threshold, masked exp-sums below the threshold split by label match, and
reduces to 4 scalars (loss, accuracy, tp, tn). Since the threshold is the
17th smallest off-diagonal distance, the selected set is exactly the 16
nearest off-diagonal columns per row; the label mask only matters on those
16 elements.

Sharding: rows data-parallel across 8 cores (1024 rows each); every core
holds all N column embeddings.

Single-scan z-encoding (the whole trick):
  For row r, col c the PE computes psum = S*(x_r . x_c) with S=512 baked
  into the row block. One DVE scalar_tensor_tensor does
      z = (psum + 2^24) + zoff[r, c]
  Adding 2^24 rounds S*(x.x) to EVEN integers (f32 RNE, ulp=2), i.e.
  2*round(256*x.x). The host-precomputed zoff table holds
      samelabel(r,c) - 2*round(128*||x_c||^2) - 2^24 + 2^22 - 2^20*diag(r,c)
  (all f32-exact integers), so z = 2^22 + 2*(q - w) + samelabel_bit exactly:
  ordering by z == ordering by squared distance (quantized to 1/128), with
  the label-match bit riding in the LSB. A single hierarchical max8 scan
  per row then yields the top-16 nearest columns TOGETHER with their label
  bits; a tiny postlude recovers d2 = ||x_r||^2 + 16384 - k/128, takes
  sqrt/exp on [128,16] tiles only, and reduces to 3 per-row stats.

The O(N) host postlude (fallback pairs, valid mask, final scalars) is
unchanged from the baseline.
"""

import numpy as np

N, D, NCORES = 8192, 128, 8
ROWS = N // NCORES          # rows per core
NRT = ROWS // 128           # row-tiles per core
NCH = 8                     # column chunks per row-tile
CHW = N // NCH              # chunk width (1024)
GRP = 512                   # stage-1 top-8 group width
S = 512.0                   # lhs scale; quantization step = 4/S in d2 units

_CACHE = {}


def _build_program():
    import concourse.mybir as mybir
    import concourse.tile as tile
    from concourse import bacc

    f32 = mybir.dt.float32
    f32r = mybir.dt.float32r
    AX = mybir.AxisListType
    OP = mybir.AluOpType
    AF = mybir.ActivationFunctionType

    nc = bacc.Bacc(
        "TRN2", target_bir_lowering=False, debug=False, num_devices=NCORES
    )

    xT_d = nc.dram_tensor("xT", [D, N], f32, kind="ExternalInput").ap()
    lhsT_d = nc.dram_tensor("lhsT", [D, ROWS], f32, kind="ExternalInput").ap()
    sqaug_d = nc.dram_tensor("sqaug", [128, NRT], f32, kind="ExternalInput").ap()
    zoff_d = nc.dram_tensor("zoff", [ROWS, N], f32, kind="ExternalInput").ap()
    outd = nc.dram_tensor("out", [128, NRT * 3], f32, kind="ExternalOutput").ap()

    TWO24 = float(2.0**24)
    TWO23 = float(2.0**23)

    with tile.TileContext(nc) as tc:
        with (
            tc.tile_pool(name="persist", bufs=1) as pp,
            tc.tile_pool(name="zoffs", bufs=4) as zp,
            tc.tile_pool(name="zs", bufs=4) as sp,
            tc.tile_pool(name="cand", bufs=2) as cp,
            tc.tile_pool(name="small", bufs=2) as smp,
            tc.tile_pool(name="psum", bufs=3, space="PSUM") as psp,
        ):
            # f32r matmul operands must be produced by a rounding engine op,
            # not a DMA: stage as f32, convert with one DVE copy each.
            xT_st = pp.tile([D, N], f32, tag="xT_st")
            nc.sync.dma_start(out=xT_st[:], in_=xT_d)
            xT_sb = pp.tile([D, N], f32r, tag="xT")
            nc.vector.tensor_copy(xT_sb[:], xT_st[:])
            lhsT_st = pp.tile([D, ROWS], f32, tag="lhsT_st")
            nc.sync.dma_start(out=lhsT_st[:], in_=lhsT_d)
            lhsT_sb = pp.tile([D, ROWS], f32r, tag="lhsT")
            nc.vector.tensor_copy(lhsT_sb[:], lhsT_st[:])
            sqaug_sb = pp.tile([128, NRT], f32, tag="sqaug")
            nc.sync.dma_start(out=sqaug_sb[:], in_=sqaug_d)

            stash = pp.tile([128, NRT * 16], f32, tag="stash")
            out_sb = pp.tile([128, NRT * 3], f32, tag="out")

            ngrp = N // GRP  # stage-1 groups per row (16)
            for rt in range(NRT):
                ce = cp.tile([128, ngrp * 8], f32, tag="ce")
                for ch in range(NCH):
                    ps = psp.tile([128, CHW], f32, tag="ps")
                    for h in range(CHW // 512):
                        c0 = ch * CHW + h * 512
                        nc.tensor.matmul(
                            ps[:, h * 512 : (h + 1) * 512],
                            lhsT_sb[:, rt * 128 : (rt + 1) * 128],
                            xT_sb[:, c0 : c0 + 512],
                            start=True,
                            stop=True,
                        )
                    zo = zp.tile([128, CHW], f32, tag="zo")
                    nc.sync.dma_start(
                        out=zo[:],
                        in_=zoff_d[
                            rt * 128 : (rt + 1) * 128, ch * CHW : (ch + 1) * CHW
                        ],
                    )
                    # z = (psum + 2^24) + zoff : RNE at ulp=2 rounds psum to
                    # even ints; zoff restores the offset and adds the
                    # label bit / column norm / diagonal penalty.
                    zt = sp.tile([128, CHW], f32, tag="zt")
                    nc.vector.scalar_tensor_tensor(
                        zt[:], ps[:], TWO24, zo[:], op0=OP.add, op1=OP.add
                    )
                    for g in range(CHW // GRP):
                        gi = ch * (CHW // GRP) + g
                        nc.vector.max(
                            ce[:, gi * 8 : gi * 8 + 8],
                            zt[:, g * GRP : (g + 1) * GRP],
                        )

                # stage 2: exact top-16 from the 128 candidates
                o = rt * 16
                ce2 = smp.tile([128, ngrp * 8], f32, tag="ce2")
                nc.vector.max(stash[:, o : o + 8], ce[:])
                nc.vector.match_replace(
                    out=ce2[:], in_to_replace=stash[:, o : o + 8], in_values=ce[:],
                    imm_value=0.0,
                )
                nc.vector.max(stash[:, o + 8 : o + 16], ce2[:])

            # device postlude on the [128, 128] stash of top-16 z values.
            # a = z/2 - 0.25 (exact); k = RNE(a + 2^23) - 2^23 recovers the
            # integer half (both z/2-0.25 and z/2+0.25 round to k at ulp=1);
            # bit = z - 2k.
            a_t = pp.tile([128, NRT * 16], f32, tag="a")
            nc.vector.tensor_scalar(
                a_t[:], stash[:], 0.5, 0.25, op0=OP.mult, op1=OP.subtract
            )
            k_t = pp.tile([128, NRT * 16], f32, tag="k")
            nc.vector.tensor_scalar(
                k_t[:], a_t[:], TWO23, TWO23, op0=OP.add, op1=OP.subtract
            )
            bit_t = pp.tile([128, NRT * 16], f32, tag="bit")
            nc.vector.scalar_tensor_tensor(
                bit_t[:], k_t[:], -2.0, stash[:], op0=OP.mult, op1=OP.add
            )
            # d2 = sqaug - k/(S/4); dist = sqrt(d2); e = exp(-dist)
            dist_t = pp.tile([128, NRT * 16], f32, tag="dist")
            for rt in range(NRT):
                o = rt * 16
                nc.scalar.activation(
                    dist_t[:, o : o + 16],
                    k_t[:, o : o + 16],
                    AF.Sqrt,
                    bias=sqaug_sb[:, rt : rt + 1],
                    scale=-4.0 / S,
                )
            e_t = pp.tile([128, NRT * 16], f32, tag="e")
            nc.scalar.activation(e_t[:], dist_t[:], AF.Exp, scale=-1.0)
            ep_t = pp.tile([128, NRT * 16], f32, tag="ep")
            nc.vector.tensor_mul(ep_t[:], e_t[:], bit_t[:])
            for rt in range(NRT):
                o = rt * 16
                nc.vector.tensor_reduce(
                    out_sb[:, rt * 3 : rt * 3 + 1], e_t[:, o : o + 16],
                    axis=AX.X, op=OP.add,
                )
                nc.vector.tensor_reduce(
                    out_sb[:, rt * 3 + 1 : rt * 3 + 2], ep_t[:, o : o + 16],
                    axis=AX.X, op=OP.add,
                )
                nc.vector.tensor_reduce(
                    out_sb[:, rt * 3 + 2 : rt * 3 + 3], bit_t[:, o : o + 16],
                    axis=AX.X, op=OP.add,
                )

            nc.sync.dma_start(out=outd, in_=out_sb[:])

    nc.compile()
    return nc


def _host_inputs(X, T):
    """Per-core input dicts. Core c's columns are rotated by c*ROWS."""
    X = X.astype(np.float32)
    sq = np.sum(X * X, axis=1)  # f32 [N]
    w2 = 2.0 * np.round((S / 4) * sq.astype(np.float64))  # even integers
    Tl = T.astype(np.int64)
    in_maps = []
    base = (-(2.0**24) + 2.0**22 - w2).astype(np.float64)  # [N] per column
    diag = np.float32(2.0**20)
    for c in range(NCORES):
        rot = np.roll(np.arange(N), -c * ROWS)
        rows = slice(c * ROWS, (c + 1) * ROWS)
        sm = (Tl[rows, None] == Tl[rot][None, :]).astype(np.float64)
        zoff = (sm + base[rot][None, :]).astype(np.float32)
        idx = np.arange(ROWS)
        zoff[idx, idx] -= diag
        in_maps.append(
            {
                "xT": np.ascontiguousarray(X[rot].T),
                "lhsT": np.ascontiguousarray((S * X[rows]).T),
                "sqaug": np.ascontiguousarray(
                    (sq[rows] + np.float32(2.0**21 / (S / 4)))
                    .reshape(NRT, 128)
                    .T
                ),
                "zoff": zoff,
            }
        )
    return in_maps


def _postlude(X, T, s_tot, s_pos, cnt_pos):
    """Host finish: fallback pairs, valid mask, final 4 scalars."""
    n = N
    Xf = X.astype(np.float64)
    sq = np.sum(X.astype(np.float32) * X.astype(np.float32), axis=1).astype(
        np.float64
    )

    cnt_pos = np.round(cnt_pos).astype(np.int64)
    count_neg = 16 - cnt_pos
    neg_logit = s_tot.astype(np.float64) - s_pos.astype(np.float64)
    neg_logit = np.maximum(neg_logit, 0.0)

    # first same-label off-diagonal index per row (order of original columns)
    first_pos = np.zeros(n, dtype=np.int64)
    order = np.argsort(T, kind="stable")
    from collections import defaultdict

    by_label = defaultdict(list)
    for idx in order:
        by_label[int(T[idx])].append(int(idx))
    for i in range(n):
        lst = by_label[int(T[i])]
        if len(lst) >= 2:
            first_pos[i] = lst[1] if lst[0] == i else lst[0]
        else:
            first_pos[i] = 0  # no positives; row is invalid anyway

    j = first_pos
    d2 = sq + sq[j] - 2.0 * np.einsum("ij,ij->i", Xf, Xf[j])
    fb_dist = np.sqrt(np.maximum(d2, 1e-12))
    fallback = np.exp(-fb_dist)

    counts = np.bincount(T.astype(np.int64), minlength=128)
    same_cnt = counts[T.astype(np.int64)] - 1
    valid = (same_cnt > 0) & ((n - 1 - same_cnt) > 0)

    pos_eff = np.where(cnt_pos == 0, fallback, s_pos.astype(np.float64))
    loss_i = -np.log(pos_eff / (pos_eff + neg_logit))
    loss = np.sum(np.where(valid, loss_i, 0.0)) / n

    count_pos_acc = np.where(cnt_pos == 0, 1, cnt_pos)
    accuracy = np.sum((valid & (count_pos_acc > count_neg)).astype(np.float64)) / n
    tp = np.sum(np.where(valid, cnt_pos, 0)) / n
    tn = np.sum(np.where(valid, count_neg, 0)) / n
    return (
        np.float32(loss),
        np.float32(accuracy),
        np.float32(tp),
        np.float32(tn),
    )


def kernel(inputs, targets):
    from concourse.bass_utils import run_bass_kernel_spmd

    X = np.asarray(inputs, dtype=np.float32)
    T = np.asarray(targets).astype(np.int64)

    if "nc" not in _CACHE:
        _CACHE["nc"] = _build_program()
    nc = _CACHE["nc"]

    in_maps = _host_inputs(X, T)
    res = run_bass_kernel_spmd(nc, in_maps, core_ids=list(range(NCORES)))

    s_tot = np.zeros(N, dtype=np.float64)
    s_pos = np.zeros(N, dtype=np.float64)
    cnt_pos = np.zeros(N, dtype=np.float64)
    for c in range(NCORES):
        out = res.results[c]["out"]  # [128, NRT*3]
        for rt in range(NRT):
            g = slice(c * ROWS + rt * 128, c * ROWS + (rt + 1) * 128)
            s_tot[g] = out[:, rt * 3]
            s_pos[g] = out[:, rt * 3 + 1]
            cnt_pos[g] = out[:, rt * 3 + 2]

    return _postlude(X, T, s_tot, s_pos, cnt_pos)


# revision 5
# speedup vs baseline: 5.8170x; 1.1986x over previous
"""KNN-softmax loss kernel for Trainium2, SPMD over 8 NeuronCores.

Problem: N=8192 points, D=128, 128 classes, K=16, alpha=1.
The reference's per-row threshold (17th smallest off-diagonal distance)
means the selected set is exactly the 16 nearest off-diagonal columns per
row; the label mask only matters on those 16 elements.

Sharding: rows data-parallel across 8 cores (1024 rows each); every core
holds all N column embeddings.

fp16 single-scan z-encoding:
  PE computes psum = A*(x_r . x_c), A=16, baked into the row block.
  The scalar engine rounds: r1 = fp16(psum + 3072) -- RNE into [2048,4096)
  where the fp16 ulp is 2, so r1 = 3072 + 2*round(8*x.x) exactly (even).
  One DVE scalar_tensor_tensor (all fp16, 2x packed) forms
      z = (r1 - 1872) + zoff[r, c]
  with the host table zoff = sgn_c*samelabel - 2*round(4*||x_c||^2)
  (fp16-exact integers; sgn_c a fixed per-column +-1 dither that cancels
  the tie-break bias; diagonal entries get -8000). So
      z = 1200 + 2*(round(8 x.x) - w_c) + sgn_c*samelabel
  exactly: ordering by z == ordering by squared distance (quantized to
  1/4 z-units = 0.25 d2... step 0.125 in d2 per z-ulp... z-step 2 = 0.25
  d2), with the label-match bit in the LSB. A single hierarchical fp16
  max8 scan yields the top-16 nearest columns together with their label
  bits. A tiny postlude recovers k via an RNE trick, bit = z - 2k,
  d2 = ||x_r||^2 + 150 - k/4, and sqrt/exp on [128,16] tiles only.

The O(N) host postlude (fallback pairs, valid mask, final scalars) is
unchanged from the baseline.
"""

import numpy as np

N, D, NCORES = 8192, 128, 8
ROWS = N // NCORES          # rows per core
NRT = ROWS // 128           # row-tiles per core
NCH = 8                     # column chunks per row-tile
CHW = N // NCH              # chunk width (1024)
GRP = 512                   # stage-1 top-8 group width
A = 16.0                    # lhs scale

_CACHE = {}


def _build_program():
    import concourse.mybir as mybir
    import concourse.tile as tile
    from concourse import bacc

    f32 = mybir.dt.float32
    f32r = mybir.dt.float32r
    f16 = mybir.dt.float16
    AX = mybir.AxisListType
    OP = mybir.AluOpType
    AF = mybir.ActivationFunctionType

    nc = bacc.Bacc(
        "TRN2", target_bir_lowering=False, debug=False, num_devices=NCORES
    )

    xT_d = nc.dram_tensor("xT", [D, N], f32, kind="ExternalInput").ap()
    lhsT_d = nc.dram_tensor("lhsT", [D, ROWS], f32, kind="ExternalInput").ap()
    sqaug_d = nc.dram_tensor("sqaug", [128, NRT], f32, kind="ExternalInput").ap()
    zoff_d = nc.dram_tensor("zoff", [ROWS, N], f16, kind="ExternalInput").ap()
    outd = nc.dram_tensor("out", [128, NRT * 3], f32, kind="ExternalOutput").ap()

    TWO23 = float(2.0**23)

    with tile.TileContext(nc) as tc:
        with (
            tc.tile_pool(name="persist", bufs=1) as pp,
            tc.tile_pool(name="zoffs", bufs=4) as zp,
            tc.tile_pool(name="rs", bufs=4) as rp,
            tc.tile_pool(name="zs", bufs=4) as sp,
            tc.tile_pool(name="cand", bufs=2) as cp,
            tc.tile_pool(name="small", bufs=2) as smp,
            tc.tile_pool(name="psum", bufs=3, space="PSUM") as psp,
        ):
            # f32r matmul operands must come from a rounding engine op, not
            # a DMA: stage as f32, convert with DVE copies. xT is staged and
            # converted in 1024-wide slices so the first matmul only waits
            # on the first slice.
            lhsT_st = pp.tile([D, ROWS], f32, tag="lhsT_st")
            nc.sync.dma_start(out=lhsT_st[:], in_=lhsT_d)
            lhsT_sb = pp.tile([D, ROWS], f32r, tag="lhsT")
            nc.vector.tensor_copy(lhsT_sb[:], lhsT_st[:])
            sqaug_sb = pp.tile([128, NRT], f32, tag="sqaug")
            nc.sync.dma_start(out=sqaug_sb[:], in_=sqaug_d)

            xT_st = pp.tile([D, N], f32, tag="xT_st")
            xT_sb = pp.tile([D, N], f32r, tag="xT")
            for j in range(NCH):
                sl = slice(j * CHW, (j + 1) * CHW)
                nc.sync.dma_start(out=xT_st[:, sl], in_=xT_d[:, sl])
                nc.vector.tensor_copy(xT_sb[:, sl], xT_st[:, sl])

            stash = pp.tile([128, NRT * 16], f16, tag="stash")
            out_sb = pp.tile([128, NRT * 3], f32, tag="out")

            ngrp = N // GRP  # stage-1 groups per row (16)
            for rt in range(NRT):
                ce = cp.tile([128, ngrp * 8], f16, tag="ce")
                for ch in range(NCH):
                    ps = psp.tile([128, CHW], f32, tag="ps")
                    for h in range(CHW // 512):
                        c0 = ch * CHW + h * 512
                        nc.tensor.matmul(
                            ps[:, h * 512 : (h + 1) * 512],
                            lhsT_sb[:, rt * 128 : (rt + 1) * 128],
                            xT_sb[:, c0 : c0 + 512],
                            start=True,
                            stop=True,
                        )
                    zo = zp.tile([128, CHW], f16, tag="zo")
                    nc.sync.dma_start(
                        out=zo[:],
                        in_=zoff_d[
                            rt * 128 : (rt + 1) * 128, ch * CHW : (ch + 1) * CHW
                        ],
                    )
                    # scalar engine: r1 = fp16(psum + 3072) -- RNE at ulp=2
                    # rounds A*x.x to even integers
                    r1 = rp.tile([128, CHW], f16, tag="r1")
                    nc.scalar.activation(
                        r1[:], ps[:], AF.Copy, bias=3072.0, scale=1.0
                    )
                    # z = (r1 - 1872) + zoff   (all fp16, packed 2x)
                    zt = sp.tile([128, CHW], f16, tag="zt")
                    nc.vector.scalar_tensor_tensor(
                        zt[:], r1[:], -1872.0, zo[:], op0=OP.add, op1=OP.add
                    )
                    for g in range(CHW // GRP):
                        gi = ch * (CHW // GRP) + g
                        nc.vector.max(
                            ce[:, gi * 8 : gi * 8 + 8],
                            zt[:, g * GRP : (g + 1) * GRP],
                        )

                # stage 2: exact top-16 from the 128 candidates
                o = rt * 16
                ce2 = smp.tile([128, ngrp * 8], f16, tag="ce2")
                nc.vector.max(stash[:, o : o + 8], ce[:])
                nc.vector.match_replace(
                    out=ce2[:], in_to_replace=stash[:, o : o + 8], in_values=ce[:],
                    imm_value=-30000.0,
                )
                nc.vector.max(stash[:, o + 8 : o + 16], ce2[:])

            # device postlude on the [128, 128] stash of top-16 z values.
            stashf = pp.tile([128, NRT * 16], f32, tag="stashf")
            nc.vector.tensor_copy(stashf[:], stash[:])
            # a = z/2 - 0.25 (exact); k = RNE(a + 2^23) - 2^23; bit = z - 2k
            a_t = pp.tile([128, NRT * 16], f32, tag="a")
            nc.vector.tensor_scalar(
                a_t[:], stashf[:], 0.5, 0.25, op0=OP.mult, op1=OP.subtract
            )
            k_t = pp.tile([128, NRT * 16], f32, tag="k")
            nc.vector.tensor_scalar(
                k_t[:], a_t[:], TWO23, TWO23, op0=OP.add, op1=OP.subtract
            )
            bit_t = pp.tile([128, NRT * 16], f32, tag="bit")
            nc.vector.scalar_tensor_tensor(
                bit_t[:], k_t[:], -2.0, stashf[:], op0=OP.mult, op1=OP.add
            )
            # d2 = sqaug - k/(A/4); dist = sqrt(d2); e = exp(-dist)
            dist_t = pp.tile([128, NRT * 16], f32, tag="dist")
            for rt in range(NRT):
                o = rt * 16
                nc.scalar.activation(
                    dist_t[:, o : o + 16],
                    k_t[:, o : o + 16],
                    AF.Sqrt,
                    bias=sqaug_sb[:, rt : rt + 1],
                    scale=-4.0 / A,
                )
            e_t = pp.tile([128, NRT * 16], f32, tag="e")
            nc.scalar.activation(e_t[:], dist_t[:], AF.Exp, scale=-1.0)
            ep_t = pp.tile([128, NRT * 16], f32, tag="ep")
            nc.vector.tensor_mul(ep_t[:], e_t[:], bit_t[:])
            for rt in range(NRT):
                o = rt * 16
                nc.vector.tensor_reduce(
                    out_sb[:, rt * 3 : rt * 3 + 1], e_t[:, o : o + 16],
                    axis=AX.X, op=OP.add,
                )
                nc.vector.tensor_reduce(
                    out_sb[:, rt * 3 + 1 : rt * 3 + 2], ep_t[:, o : o + 16],
                    axis=AX.X, op=OP.add,
                )
                nc.vector.tensor_reduce(
                    out_sb[:, rt * 3 + 2 : rt * 3 + 3], bit_t[:, o : o + 16],
                    axis=AX.X, op=OP.add,
                )

            nc.sync.dma_start(out=outd, in_=out_sb[:])

    nc.compile()
    return nc


def _host_inputs(X, T):
    """Per-core input dicts. Core c's columns are rotated by c*ROWS."""
    X = X.astype(np.float32)
    sq = np.sum(X * X, axis=1)  # f32 [N]
    w2 = 2.0 * np.minimum(np.round((A / 4) * sq.astype(np.float64)), 1023.0)
    rng = np.random.default_rng(12345)
    sgn = rng.integers(0, 2, size=N).astype(np.float64) * 2.0 - 1.0
    Tl = T.astype(np.int64)
    in_maps = []
    for c in range(NCORES):
        rot = np.roll(np.arange(N), -c * ROWS)
        rows = slice(c * ROWS, (c + 1) * ROWS)
        sm = (Tl[rows, None] == Tl[rot][None, :]).astype(np.float64)
        zoff = (sgn[rot][None, :] * sm - w2[rot][None, :]).astype(np.float16)
        idx = np.arange(ROWS)
        zoff[idx, idx] = np.float16(-8000.0)
        in_maps.append(
            {
                "xT": np.ascontiguousarray(X[rot].T),
                "lhsT": np.ascontiguousarray((A * X[rows]).T),
                "sqaug": np.ascontiguousarray(
                    (sq[rows] + np.float32(150.0)).reshape(NRT, 128).T
                ),
                "zoff": zoff,
            }
        )
    return in_maps


def _postlude(X, T, s_tot, s_pos, cnt_pos):
    """Host finish: fallback pairs, valid mask, final 4 scalars."""
    n = N
    Xf = X.astype(np.float64)
    sq = np.sum(X.astype(np.float32) * X.astype(np.float32), axis=1).astype(
        np.float64
    )

    cnt_pos = np.round(cnt_pos).astype(np.int64)
    count_neg = 16 - cnt_pos
    neg_logit = s_tot.astype(np.float64) - s_pos.astype(np.float64)
    neg_logit = np.maximum(neg_logit, 0.0)

    # first same-label off-diagonal index per row (order of original columns)
    first_pos = np.zeros(n, dtype=np.int64)
    order = np.argsort(T, kind="stable")
    from collections import defaultdict

    by_label = defaultdict(list)
    for idx in order:
        by_label[int(T[idx])].append(int(idx))
    for i in range(n):
        lst = by_label[int(T[i])]
        if len(lst) >= 2:
            first_pos[i] = lst[1] if lst[0] == i else lst[0]
        else:
            first_pos[i] = 0  # no positives; row is invalid anyway

    j = first_pos
    d2 = sq + sq[j] - 2.0 * np.einsum("ij,ij->i", Xf, Xf[j])
    fb_dist = np.sqrt(np.maximum(d2, 1e-12))
    fallback = np.exp(-fb_dist)

    counts = np.bincount(T.astype(np.int64), minlength=128)
    same_cnt = counts[T.astype(np.int64)] - 1
    valid = (same_cnt > 0) & ((n - 1 - same_cnt) > 0)

    pos_eff = np.where(cnt_pos == 0, fallback, s_pos.astype(np.float64))
    loss_i = -np.log(pos_eff / (pos_eff + neg_logit))
    loss = np.sum(np.where(valid, loss_i, 0.0)) / n

    count_pos_acc = np.where(cnt_pos == 0, 1, cnt_pos)
    accuracy = np.sum((valid & (count_pos_acc > count_neg)).astype(np.float64)) / n
    tp = np.sum(np.where(valid, cnt_pos, 0)) / n
    tn = np.sum(np.where(valid, count_neg, 0)) / n
    return (
        np.float32(loss),
        np.float32(accuracy),
        np.float32(tp),
        np.float32(tn),
    )


def kernel(inputs, targets):
    from concourse.bass_utils import run_bass_kernel_spmd

    X = np.asarray(inputs, dtype=np.float32)
    T = np.asarray(targets).astype(np.int64)

    if "nc" not in _CACHE:
        _CACHE["nc"] = _build_program()
    nc = _CACHE["nc"]

    in_maps = _host_inputs(X, T)
    res = run_bass_kernel_spmd(nc, in_maps, core_ids=list(range(NCORES)))

    s_tot = np.zeros(N, dtype=np.float64)
    s_pos = np.zeros(N, dtype=np.float64)
    cnt_pos = np.zeros(N, dtype=np.float64)
    for c in range(NCORES):
        out = res.results[c]["out"]  # [128, NRT*3]
        for rt in range(NRT):
            g = slice(c * ROWS + rt * 128, c * ROWS + (rt + 1) * 128)
            s_tot[g] = out[:, rt * 3]
            s_pos[g] = out[:, rt * 3 + 1]
            cnt_pos[g] = out[:, rt * 3 + 2]

    return _postlude(X, T, s_tot, s_pos, cnt_pos)


# revision 6
# speedup vs baseline: 5.9745x; 1.0271x over previous
"""KNN-softmax loss kernel for Trainium2, SPMD over 8 NeuronCores.

Problem: N=8192 points, D=128, 128 classes, K=16, alpha=1.
The reference's per-row threshold (17th smallest off-diagonal distance)
means the selected set is exactly the 16 nearest off-diagonal columns per
row; the label mask only matters on those 16 elements.

Sharding: rows data-parallel across 8 cores (1024 rows each); every core
holds all N column embeddings.

fp16 single-scan z-encoding:
  PE computes psum = A*(x_r . x_c), A=16, baked into the row block.
  The scalar engine rounds: r1 = fp16(psum + 3072) -- RNE into [2048,4096)
  where the fp16 ulp is 2, so r1 = 3072 + 2*round(8*x.x) exactly (even).
  One DVE scalar_tensor_tensor (all fp16) forms
      z = (r1 - 1872) + zoff[r, c]
  with the host table zoff = sgn_c*samelabel - 2*round(4*||x_c||^2)
  (fp16-exact integers; sgn_c a fixed per-column +-1 dither that cancels
  the tie-break bias; diagonal entries get -8000). So
      z = 1200 + 2*(round(8 x.x) - w_c) + sgn_c*samelabel
  exactly: ordering by z == ordering by negated squared distance
  (quantized to 0.125 d2 units), with the label-match bit in the LSB.
  A single hierarchical fp16 max8 scan (top-8 per 1024-group, then
  top-16 of 64 candidates) yields the 16 nearest columns together with
  their label bits. A tiny postlude recovers k via an RNE trick,
  bit = z - 2k, d2 = ||x_r||^2 + 150 - k/4, and sqrt/exp on [128,16]
  tiles only.

The O(N) host postlude (fallback pairs, valid mask, final scalars) is
unchanged from the baseline.
"""

import numpy as np

N, D, NCORES = 8192, 128, 8
ROWS = N // NCORES          # rows per core
NRT = ROWS // 128           # row-tiles per core
NCH = 4                     # column chunks per row-tile
CHW = N // NCH              # chunk width (2048)
GRP = 1024                  # stage-1 top-8 group width
A = 16.0                    # lhs scale

_CACHE = {}


def _build_program(f32r_dram):
    import concourse.mybir as mybir
    import concourse.tile as tile
    from concourse import bacc

    f32 = mybir.dt.float32
    f32r = mybir.dt.float32r
    f16 = mybir.dt.float16
    AX = mybir.AxisListType
    OP = mybir.AluOpType
    AF = mybir.ActivationFunctionType

    nc = bacc.Bacc(
        "TRN2", target_bir_lowering=False, debug=False, num_devices=NCORES
    )

    xdt = f32r if f32r_dram else f32
    xT_d = nc.dram_tensor("xT", [D, N], xdt, kind="ExternalInput").ap()
    lhsT_d = nc.dram_tensor("lhsT", [D, ROWS], xdt, kind="ExternalInput").ap()
    sqaug_d = nc.dram_tensor("sqaug", [128, NRT], f32, kind="ExternalInput").ap()
    zoff_d = nc.dram_tensor("zoff", [ROWS, N], f16, kind="ExternalInput").ap()
    outd = nc.dram_tensor("out", [128, NRT * 3], f32, kind="ExternalOutput").ap()

    TWO23 = float(2.0**23)

    with tile.TileContext(nc) as tc:
        with (
            tc.tile_pool(name="persist", bufs=1) as pp,
            tc.tile_pool(name="zoffs", bufs=4) as zp,
            tc.tile_pool(name="rs", bufs=4) as rp,
            tc.tile_pool(name="zs", bufs=4) as sp,
            tc.tile_pool(name="cand", bufs=2) as cp,
            tc.tile_pool(name="small", bufs=2) as smp,
            tc.tile_pool(name="psum", bufs=2, space="PSUM") as psp,
        ):
            sqaug_sb = pp.tile([128, NRT], f32, tag="sqaug")
            nc.sync.dma_start(out=sqaug_sb[:], in_=sqaug_d)
            lhsT_sb = pp.tile([D, ROWS], f32r, tag="lhsT")
            xT_sb = pp.tile([D, N], f32r, tag="xT")
            if f32r_dram:
                nc.sync.dma_start(out=lhsT_sb[:], in_=lhsT_d)
                for j in range(8):
                    sl = slice(j * 1024, (j + 1) * 1024)
                    nc.sync.dma_start(out=xT_sb[:, sl], in_=xT_d[:, sl])
            else:
                # f32r matmul operands must come from a rounding engine op,
                # not a DMA: stage as f32, convert with DVE copies (sliced
                # so the first matmul only waits on the first slice).
                lhsT_st = pp.tile([D, ROWS], f32, tag="lhsT_st")
                nc.sync.dma_start(out=lhsT_st[:], in_=lhsT_d)
                nc.vector.tensor_copy(lhsT_sb[:], lhsT_st[:])
                xT_st = pp.tile([D, N], f32, tag="xT_st")
                for j in range(8):
                    sl = slice(j * 1024, (j + 1) * 1024)
                    nc.sync.dma_start(out=xT_st[:, sl], in_=xT_d[:, sl])
                    nc.vector.tensor_copy(xT_sb[:, sl], xT_st[:, sl])

            stash = pp.tile([128, NRT * 16], f16, tag="stash")
            out_sb = pp.tile([128, NRT * 3], f32, tag="out")

            ngrp = N // GRP  # stage-1 groups per row (8)
            for rt in range(NRT):
                ce = cp.tile([128, ngrp * 8], f16, tag="ce")
                for ch in range(NCH):
                    ps = psp.tile([128, CHW], f32, tag="ps")
                    for h in range(CHW // 512):
                        c0 = ch * CHW + h * 512
                        nc.tensor.matmul(
                            ps[:, h * 512 : (h + 1) * 512],
                            lhsT_sb[:, rt * 128 : (rt + 1) * 128],
                            xT_sb[:, c0 : c0 + 512],
                            start=True,
                            stop=True,
                        )
                    zo = zp.tile([128, CHW], f16, tag="zo")
                    nc.sync.dma_start(
                        out=zo[:],
                        in_=zoff_d[
                            rt * 128 : (rt + 1) * 128, ch * CHW : (ch + 1) * CHW
                        ],
                    )
                    # scalar engine: r1 = fp16(psum + 3072) -- RNE at ulp=2
                    # rounds A*x.x to even integers
                    r1 = rp.tile([128, CHW], f16, tag="r1")
                    nc.scalar.activation(
                        r1[:], ps[:], AF.Copy, bias=3072.0, scale=1.0
                    )
                    # z = (r1 - 1872) + zoff   (all fp16)
                    zt = sp.tile([128, CHW], f16, tag="zt")
                    nc.vector.scalar_tensor_tensor(
                        zt[:], r1[:], -1872.0, zo[:], op0=OP.add, op1=OP.add
                    )
                    for g in range(CHW // GRP):
                        gi = ch * (CHW // GRP) + g
                        nc.vector.max(
                            ce[:, gi * 8 : gi * 8 + 8],
                            zt[:, g * GRP : (g + 1) * GRP],
                        )

                # stage 2: exact top-16 from the 64 candidates
                o = rt * 16
                ce2 = smp.tile([128, ngrp * 8], f16, tag="ce2")
                nc.vector.max(stash[:, o : o + 8], ce[:])
                nc.vector.match_replace(
                    out=ce2[:], in_to_replace=stash[:, o : o + 8], in_values=ce[:],
                    imm_value=-30000.0,
                )
                nc.vector.max(stash[:, o + 8 : o + 16], ce2[:])

            # device postlude on the [128, 128] stash of top-16 z values.
            stashf = pp.tile([128, NRT * 16], f32, tag="stashf")
            nc.vector.tensor_copy(stashf[:], stash[:])
            # a = z/2 - 0.25 (exact); k = RNE(a + 2^23) - 2^23; bit = z - 2k
            a_t = pp.tile([128, NRT * 16], f32, tag="a")
            nc.vector.tensor_scalar(
                a_t[:], stashf[:], 0.5, 0.25, op0=OP.mult, op1=OP.subtract
            )
            k_t = pp.tile([128, NRT * 16], f32, tag="k")
            nc.vector.tensor_scalar(
                k_t[:], a_t[:], TWO23, TWO23, op0=OP.add, op1=OP.subtract
            )
            bit_t = pp.tile([128, NRT * 16], f32, tag="bit")
            nc.vector.scalar_tensor_tensor(
                bit_t[:], k_t[:], -2.0, stashf[:], op0=OP.mult, op1=OP.add
            )
            # d2 = sqaug - k/(A/4); dist = sqrt(d2); e = exp(-dist)
            dist_t = pp.tile([128, NRT * 16], f32, tag="dist")
            for rt in range(NRT):
                o = rt * 16
                nc.scalar.activation(
                    dist_t[:, o : o + 16],
                    k_t[:, o : o + 16],
                    AF.Sqrt,
                    bias=sqaug_sb[:, rt : rt + 1],
                    scale=-4.0 / A,
                )
            ep_t = pp.tile([128, NRT * 16], f32, tag="ep")
            for rt in range(NRT):
                o = rt * 16
                # e = exp(-dist), s_tot accumulated by the scalar engine
                e_sl = ep_t[:, o : o + 16]
                nc.scalar.activation(
                    e_sl, dist_t[:, o : o + 16], AF.Exp, scale=-1.0,
                    accum_out=out_sb[:, rt * 3 : rt * 3 + 1],
                )
            for rt in range(NRT):
                o = rt * 16
                # s_pos = sum(e*bit) fused via accum_out
                junk = smp.tile([128, 16], f32, tag="junk")
                nc.vector.scalar_tensor_tensor(
                    junk[:], ep_t[:, o : o + 16], 1.0, bit_t[:, o : o + 16],
                    op0=OP.mult, op1=OP.mult,
                    accum_out=out_sb[:, rt * 3 + 1 : rt * 3 + 2],
                )
                nc.vector.tensor_reduce(
                    out_sb[:, rt * 3 + 2 : rt * 3 + 3], bit_t[:, o : o + 16],
                    axis=AX.X, op=OP.add,
                )

            nc.sync.dma_start(out=outd, in_=out_sb[:])

    nc.compile()
    return nc


def _host_inputs(X, T):
    """Per-core input dicts. Core c's columns are rotated by c*ROWS."""
    X = X.astype(np.float32)
    sq = np.sum(X * X, axis=1)  # f32 [N]
    w2 = 2.0 * np.minimum(np.round((A / 4) * sq.astype(np.float64)), 1023.0)
    rng = np.random.default_rng(12345)
    sgn = rng.integers(0, 2, size=N).astype(np.float64) * 2.0 - 1.0
    Tl = T.astype(np.int64)
    in_maps = []
    for c in range(NCORES):
        rot = np.roll(np.arange(N), -c * ROWS)
        rows = slice(c * ROWS, (c + 1) * ROWS)
        sm = (Tl[rows, None] == Tl[rot][None, :]).astype(np.float64)
        zoff = (sgn[rot][None, :] * sm - w2[rot][None, :]).astype(np.float16)
        idx = np.arange(ROWS)
        zoff[idx, idx] = np.float16(-8000.0)
        in_maps.append(
            {
                "xT": np.ascontiguousarray(X[rot].T),
                "lhsT": np.ascontiguousarray((A * X[rows]).T),
                "sqaug": np.ascontiguousarray(
                    (sq[rows] + np.float32(150.0)).reshape(NRT, 128).T
                ),
                "zoff": zoff,
            }
        )
    return in_maps


def _postlude(X, T, s_tot, s_pos, cnt_pos):
    """Host finish: fallback pairs, valid mask, final 4 scalars."""
    n = N
    Xf = X.astype(np.float64)
    sq = np.sum(X.astype(np.float32) * X.astype(np.float32), axis=1).astype(
        np.float64
    )

    cnt_pos = np.round(cnt_pos).astype(np.int64)
    count_neg = 16 - cnt_pos
    neg_logit = s_tot.astype(np.float64) - s_pos.astype(np.float64)
    neg_logit = np.maximum(neg_logit, 0.0)

    # first same-label off-diagonal index per row (order of original columns)
    first_pos = np.zeros(n, dtype=np.int64)
    order = np.argsort(T, kind="stable")
    from collections import defaultdict

    by_label = defaultdict(list)
    for idx in order:
        by_label[int(T[idx])].append(int(idx))
    for i in range(n):
        lst = by_label[int(T[i])]
        if len(lst) >= 2:
            first_pos[i] = lst[1] if lst[0] == i else lst[0]
        else:
            first_pos[i] = 0  # no positives; row is invalid anyway

    j = first_pos
    d2 = sq + sq[j] - 2.0 * np.einsum("ij,ij->i", Xf, Xf[j])
    fb_dist = np.sqrt(np.maximum(d2, 1e-12))
    fallback = np.exp(-fb_dist)

    counts = np.bincount(T.astype(np.int64), minlength=128)
    same_cnt = counts[T.astype(np.int64)] - 1
    valid = (same_cnt > 0) & ((n - 1 - same_cnt) > 0)

    pos_eff = np.where(cnt_pos == 0, fallback, s_pos.astype(np.float64))
    loss_i = -np.log(pos_eff / (pos_eff + neg_logit))
    loss = np.sum(np.where(valid, loss_i, 0.0)) / n

    count_pos_acc = np.where(cnt_pos == 0, 1, cnt_pos)
    accuracy = np.sum((valid & (count_pos_acc > count_neg)).astype(np.float64)) / n
    tp = np.sum(np.where(valid, cnt_pos, 0)) / n
    tn = np.sum(np.where(valid, count_neg, 0)) / n
    return (
        np.float32(loss),
        np.float32(accuracy),
        np.float32(tp),
        np.float32(tn),
    )


def _get_nc():
    if "nc" in _CACHE:
        return _CACHE["nc"]
    _CACHE["nc"] = _build_program(f32r_dram=_CACHE.get("f32r_dram", True))
    return _CACHE["nc"]


def kernel(inputs, targets):
    from concourse.bass_utils import run_bass_kernel_spmd

    X = np.asarray(inputs, dtype=np.float32)
    T = np.asarray(targets).astype(np.int64)

    in_maps = _host_inputs(X, T)
    try:
        nc = _get_nc()
        res = run_bass_kernel_spmd(nc, in_maps, core_ids=list(range(NCORES)))
    except Exception:
        if _CACHE.get("f32r_dram", True) is False:
            raise
        # f32r-typed DRAM inputs rejected by the BIR verifier on this
        # toolchain: rebuild with f32 staging + DVE conversion.
        _CACHE.pop("nc", None)
        _CACHE["f32r_dram"] = False
        nc = _get_nc()
        res = run_bass_kernel_spmd(nc, in_maps, core_ids=list(range(NCORES)))

    s_tot = np.zeros(N, dtype=np.float64)
    s_pos = np.zeros(N, dtype=np.float64)
    cnt_pos = np.zeros(N, dtype=np.float64)
    for c in range(NCORES):
        out = res.results[c]["out"]  # [128, NRT*3]
        for rt in range(NRT):
            g = slice(c * ROWS + rt * 128, c * ROWS + (rt + 1) * 128)
            s_tot[g] = out[:, rt * 3]
            s_pos[g] = out[:, rt * 3 + 1]
            cnt_pos[g] = out[:, rt * 3 + 2]

    return _postlude(X, T, s_tot, s_pos, cnt_pos)


# revision 9
# speedup vs baseline: 6.0738x; 1.0166x over previous
"""KNN-softmax loss kernel for Trainium2, SPMD over 8 NeuronCores.

Problem: N=8192 points, D=128, 128 classes, K=16, alpha=1.
The reference's per-row threshold (17th smallest off-diagonal distance)
means the selected set is exactly the 16 nearest off-diagonal columns per
row; the label mask only matters on those 16 elements.

Sharding: rows data-parallel across 8 cores (1024 rows each); every core
holds all N column embeddings.

fp16 single-scan z-encoding:
  PE computes psum = A*(x_r . x_c), A=16, baked into the row block.
  The scalar engine rounds: r1 = fp16(psum + 3072) -- RNE into [2048,4096)
  where the fp16 ulp is 2, so r1 = 3072 + 2*round(8*x.x) exactly (even).
  One DVE scalar_tensor_tensor (all fp16) forms
      z = (r1 - 1872) + zoff[r, c]
  with the host table zoff = sgn_c*samelabel - 2*round(4*||x_c||^2)
  (fp16-exact integers; sgn_c a fixed per-column +-1 dither that cancels
  the tie-break bias; diagonal entries get -8000). So
      z = 1200 + 2*(round(8 x.x) - w_c) + sgn_c*samelabel
  exactly: ordering by z == ordering by negated squared distance
  (quantized to 0.125 d2 units), with the label-match bit in the LSB.
  A single hierarchical fp16 max8 scan (top-8 per 1024-group, then
  top-16 of 64 candidates) yields the 16 nearest columns together with
  their label bits. A tiny postlude recovers k via an RNE trick,
  bit = z - 2k, d2 = ||x_r||^2 + 150 - k/4, and sqrt/exp on [128,16]
  tiles only.

The O(N) host postlude (fallback pairs, valid mask, final scalars) is
unchanged from the baseline.
"""

import numpy as np

N, D, NCORES = 8192, 128, 8
ROWS = N // NCORES          # rows per core
NRT = ROWS // 128           # row-tiles per core
NCH = 4                     # column chunks per row-tile
CHW = N // NCH              # chunk width (2048)
GRP = 2048                  # stage-1 top-8 group width
A = 16.0                    # lhs scale

_CACHE = {}


def _build_program(f32r_dram):
    import concourse.mybir as mybir
    import concourse.tile as tile
    from concourse import bacc

    f32 = mybir.dt.float32
    f32r = mybir.dt.float32r
    f16 = mybir.dt.float16
    AX = mybir.AxisListType
    OP = mybir.AluOpType
    AF = mybir.ActivationFunctionType

    nc = bacc.Bacc(
        "TRN2", target_bir_lowering=False, debug=False, num_devices=NCORES
    )

    xdt = f32r if f32r_dram else f32
    xT_d = nc.dram_tensor("xT", [D, N], xdt, kind="ExternalInput").ap()
    lhsT_d = nc.dram_tensor("lhsT", [D, ROWS], xdt, kind="ExternalInput").ap()
    sqaug_d = nc.dram_tensor("sqaug", [128, NRT], f32, kind="ExternalInput").ap()
    zoff_d = nc.dram_tensor("zoff", [ROWS, N], f16, kind="ExternalInput").ap()
    outd = nc.dram_tensor("out", [128, NRT * 3], f32, kind="ExternalOutput").ap()

    TWO23 = float(2.0**23)

    with tile.TileContext(nc) as tc:
        with (
            tc.tile_pool(name="persist", bufs=1) as pp,
            tc.tile_pool(name="zoffs", bufs=4) as zp,
            tc.tile_pool(name="rs", bufs=4) as rp,
            tc.tile_pool(name="zs", bufs=4) as sp,
            tc.tile_pool(name="cand", bufs=2) as cp,
            tc.tile_pool(name="small", bufs=2) as smp,
            tc.tile_pool(name="psum", bufs=2, space="PSUM") as psp,
        ):
            sqaug_sb = pp.tile([128, NRT], f32, tag="sqaug")
            nc.sync.dma_start(out=sqaug_sb[:], in_=sqaug_d)
            lhsT_sb = pp.tile([D, ROWS], f32r, tag="lhsT")
            xT_sb = pp.tile([D, N], f32r, tag="xT")
            if f32r_dram:
                nc.sync.dma_start(out=lhsT_sb[:], in_=lhsT_d)
                for j in range(16):
                    sl = slice(j * 512, (j + 1) * 512)
                    nc.sync.dma_start(out=xT_sb[:, sl], in_=xT_d[:, sl])
            else:
                # f32r matmul operands must come from a rounding engine op,
                # not a DMA: stage as f32, convert with DVE copies (sliced
                # so the first matmul only waits on the first slice).
                lhsT_st = pp.tile([D, ROWS], f32, tag="lhsT_st")
                nc.sync.dma_start(out=lhsT_st[:], in_=lhsT_d)
                nc.vector.tensor_copy(lhsT_sb[:], lhsT_st[:])
                xT_st = pp.tile([D, N], f32, tag="xT_st")
                for j in range(8):
                    sl = slice(j * 1024, (j + 1) * 1024)
                    nc.sync.dma_start(out=xT_st[:, sl], in_=xT_d[:, sl])
                    nc.vector.tensor_copy(xT_sb[:, sl], xT_st[:, sl])

            out_sb = pp.tile([128, NRT * 3], f32, tag="out")

            ngrp = N // GRP  # stage-1 groups per row (4)
            for rt in range(NRT):
                ce = cp.tile([128, ngrp * 8], f16, tag="ce")
                for ch in range(NCH):
                    ps = psp.tile([128, CHW], f32, tag="ps")
                    for h in range(CHW // 512):
                        c0 = ch * CHW + h * 512
                        nc.tensor.matmul(
                            ps[:, h * 512 : (h + 1) * 512],
                            lhsT_sb[:, rt * 128 : (rt + 1) * 128],
                            xT_sb[:, c0 : c0 + 512],
                            start=True,
                            stop=True,
                        )
                    zo = zp.tile([128, CHW], f16, tag="zo")
                    nc.sync.dma_start(
                        out=zo[:],
                        in_=zoff_d[
                            rt * 128 : (rt + 1) * 128, ch * CHW : (ch + 1) * CHW
                        ],
                    )
                    # scalar engine: r1 = fp16(psum + 3072) -- RNE at ulp=2
                    # rounds A*x.x to even integers
                    r1 = rp.tile([128, CHW], f16, tag="r1")
                    nc.scalar.activation(
                        r1[:], ps[:], AF.Copy, bias=3072.0, scale=1.0
                    )
                    # z = (r1 - 1872) + zoff   (all fp16)
                    zt = sp.tile([128, CHW], f16, tag="zt")
                    nc.vector.scalar_tensor_tensor(
                        zt[:], r1[:], -1872.0, zo[:], op0=OP.add, op1=OP.add
                    )
                    nc.vector.max(ce[:, ch * 8 : ch * 8 + 8], zt[:])

                # stage 2: exact top-16 from the 32 candidates, then the
                # per-rt postlude inline so there is no serial tail.
                m16 = smp.tile([128, 16], f16, tag="m16")
                ce2 = smp.tile([128, ngrp * 8], f16, tag="ce2")
                nc.vector.max(m16[:, 0:8], ce[:])
                nc.vector.match_replace(
                    out=ce2[:], in_to_replace=m16[:, 0:8], in_values=ce[:],
                    imm_value=-30000.0,
                )
                nc.vector.max(m16[:, 8:16], ce2[:])

                # a = z/2-0.25 (exact); k = RNE(a+2^23)-2^23; bit = z-2k
                sf = smp.tile([128, 16], f32, tag="sf")
                nc.vector.tensor_copy(sf[:], m16[:])
                a_t = smp.tile([128, 16], f32, tag="a")
                nc.vector.tensor_scalar(
                    a_t[:], sf[:], 0.5, 0.25, op0=OP.mult, op1=OP.subtract
                )
                k_t = smp.tile([128, 16], f32, tag="k")
                nc.vector.tensor_scalar(
                    k_t[:], a_t[:], TWO23, TWO23, op0=OP.add, op1=OP.subtract
                )
                bit_t = smp.tile([128, 16], f32, tag="bit")
                nc.vector.scalar_tensor_tensor(
                    bit_t[:], k_t[:], -2.0, sf[:], op0=OP.mult, op1=OP.add
                )
                # d2 = sqaug - k/(A/4); dist = sqrt(d2); e = exp(-dist)
                dist_t = smp.tile([128, 16], f32, tag="dist")
                nc.scalar.activation(
                    dist_t[:], k_t[:], AF.Sqrt,
                    bias=sqaug_sb[:, rt : rt + 1], scale=-4.0 / A,
                )
                e_t = smp.tile([128, 16], f32, tag="e")
                nc.scalar.activation(
                    e_t[:], dist_t[:], AF.Exp, scale=-1.0,
                    accum_out=out_sb[:, rt * 3 : rt * 3 + 1],
                )
                junk = smp.tile([128, 16], f32, tag="junk")
                nc.vector.scalar_tensor_tensor(
                    junk[:], e_t[:], 1.0, bit_t[:],
                    op0=OP.mult, op1=OP.mult,
                    accum_out=out_sb[:, rt * 3 + 1 : rt * 3 + 2],
                )
                nc.vector.tensor_reduce(
                    out_sb[:, rt * 3 + 2 : rt * 3 + 3], bit_t[:],
                    axis=AX.X, op=OP.add,
                )

            nc.sync.dma_start(out=outd, in_=out_sb[:])

    nc.compile()
    return nc


def _host_inputs(X, T):
    """Per-core input dicts. Core c's columns are rotated by c*ROWS."""
    X = X.astype(np.float32)
    sq = np.sum(X * X, axis=1)  # f32 [N]
    w2 = 2.0 * np.minimum(np.round((A / 4) * sq.astype(np.float64)), 1023.0)
    rng = np.random.default_rng(12345)
    sgn = rng.integers(0, 2, size=N).astype(np.float64) * 2.0 - 1.0
    Tl = T.astype(np.int64)
    in_maps = []
    for c in range(NCORES):
        rot = np.roll(np.arange(N), -c * ROWS)
        rows = slice(c * ROWS, (c + 1) * ROWS)
        sm = (Tl[rows, None] == Tl[rot][None, :]).astype(np.float64)
        zoff = (sgn[rot][None, :] * sm - w2[rot][None, :]).astype(np.float16)
        idx = np.arange(ROWS)
        zoff[idx, idx] = np.float16(-8000.0)
        in_maps.append(
            {
                "xT": np.ascontiguousarray(X[rot].T),
                "lhsT": np.ascontiguousarray((A * X[rows]).T),
                "sqaug": np.ascontiguousarray(
                    (sq[rows] + np.float32(150.0)).reshape(NRT, 128).T
                ),
                "zoff": zoff,
            }
        )
    return in_maps


def _postlude(X, T, s_tot, s_pos, cnt_pos):
    """Host finish: fallback pairs, valid mask, final 4 scalars."""
    n = N
    Xf = X.astype(np.float64)
    sq = np.sum(X.astype(np.float32) * X.astype(np.float32), axis=1).astype(
        np.float64
    )

    cnt_pos = np.round(cnt_pos).astype(np.int64)
    count_neg = 16 - cnt_pos
    neg_logit = s_tot.astype(np.float64) - s_pos.astype(np.float64)
    neg_logit = np.maximum(neg_logit, 0.0)

    # first same-label off-diagonal index per row (order of original columns)
    first_pos = np.zeros(n, dtype=np.int64)
    order = np.argsort(T, kind="stable")
    from collections import defaultdict

    by_label = defaultdict(list)
    for idx in order:
        by_label[int(T[idx])].append(int(idx))
    for i in range(n):
        lst = by_label[int(T[i])]
        if len(lst) >= 2:
            first_pos[i] = lst[1] if lst[0] == i else lst[0]
        else:
            first_pos[i] = 0  # no positives; row is invalid anyway

    j = first_pos
    d2 = sq + sq[j] - 2.0 * np.einsum("ij,ij->i", Xf, Xf[j])
    fb_dist = np.sqrt(np.maximum(d2, 1e-12))
    fallback = np.exp(-fb_dist)

    counts = np.bincount(T.astype(np.int64), minlength=128)
    same_cnt = counts[T.astype(np.int64)] - 1
    valid = (same_cnt > 0) & ((n - 1 - same_cnt) > 0)

    pos_eff = np.where(cnt_pos == 0, fallback, s_pos.astype(np.float64))
    loss_i = -np.log(pos_eff / (pos_eff + neg_logit))
    loss = np.sum(np.where(valid, loss_i, 0.0)) / n

    count_pos_acc = np.where(cnt_pos == 0, 1, cnt_pos)
    accuracy = np.sum((valid & (count_pos_acc > count_neg)).astype(np.float64)) / n
    tp = np.sum(np.where(valid, cnt_pos, 0)) / n
    tn = np.sum(np.where(valid, count_neg, 0)) / n
    return (
        np.float32(loss),
        np.float32(accuracy),
        np.float32(tp),
        np.float32(tn),
    )


def _get_nc():
    if "nc" in _CACHE:
        return _CACHE["nc"]
    _CACHE["nc"] = _build_program(f32r_dram=_CACHE.get("f32r_dram", True))
    return _CACHE["nc"]


def kernel(inputs, targets):
    from concourse.bass_utils import run_bass_kernel_spmd

    X = np.asarray(inputs, dtype=np.float32)
    T = np.asarray(targets).astype(np.int64)

    in_maps = _host_inputs(X, T)
    try:
        nc = _get_nc()
        res = run_bass_kernel_spmd(nc, in_maps, core_ids=list(range(NCORES)))
    except Exception:
        if _CACHE.get("f32r_dram", True) is False:
            raise
        # f32r-typed DRAM inputs rejected by the BIR verifier on this
        # toolchain: rebuild with f32 staging + DVE conversion.
        _CACHE.pop("nc", None)
        _CACHE["f32r_dram"] = False
        nc = _get_nc()
        res = run_bass_kernel_spmd(nc, in_maps, core_ids=list(range(NCORES)))

    s_tot = np.zeros(N, dtype=np.float64)
    s_pos = np.zeros(N, dtype=np.float64)
    cnt_pos = np.zeros(N, dtype=np.float64)
    for c in range(NCORES):
        out = res.results[c]["out"]  # [128, NRT*3]
        for rt in range(NRT):
            g = slice(c * ROWS + rt * 128, c * ROWS + (rt + 1) * 128)
            s_tot[g] = out[:, rt * 3]
            s_pos[g] = out[:, rt * 3 + 1]
            cnt_pos[g] = out[:, rt * 3 + 2]

    return _postlude(X, T, s_tot, s_pos, cnt_pos)


# revision 15
# speedup vs baseline: 6.1179x; 1.0073x over previous
"""KNN-softmax loss kernel for Trainium2, SPMD over 8 NeuronCores.

Problem: N=8192 points, D=128, 128 classes, K=16, alpha=1.
The reference's per-row threshold (17th smallest off-diagonal distance)
means the selected set is exactly the 16 nearest off-diagonal columns per
row; the label mask only matters on those 16 elements.

Sharding: rows data-parallel across 8 cores (1024 rows each); every core
holds all N column embeddings.

fp16 single-scan z-encoding:
  PE computes psum = A*(x_r . x_c), A=16, baked into the row block.
  The scalar engine rounds: r1 = fp16(psum + 3072) -- RNE into [2048,4096)
  where the fp16 ulp is 2, so r1 = 3072 + 2*round(8*x.x) exactly (even).
  One DVE scalar_tensor_tensor (all fp16) forms
      z = (r1 - 1872) + zoff[r, c]
  with the host table zoff = sgn_c*samelabel - 2*round(4*||x_c||^2)
  (fp16-exact integers; sgn_c a fixed per-column +-1 dither that cancels
  the tie-break bias; diagonal entries get -8000). So
      z = 1200 + 2*(round(8 x.x) - w_c) + sgn_c*samelabel
  exactly: ordering by z == ordering by negated squared distance
  (quantized to 0.125 d2 units), with the label-match bit in the LSB.
  A single hierarchical fp16 max8 scan (top-8 per 1024-group, then
  top-16 of 64 candidates) yields the 16 nearest columns together with
  their label bits. A tiny postlude recovers k via an RNE trick,
  bit = z - 2k, d2 = ||x_r||^2 + 150 - k/4, and sqrt/exp on [128,16]
  tiles only.

The O(N) host postlude (fallback pairs, valid mask, final scalars) is
unchanged from the baseline.
"""

import numpy as np

N, D, NCORES = 8192, 128, 8
ROWS = N // NCORES          # rows per core
NRT = ROWS // 128           # row-tiles per core
NCH = 4                     # column chunks per row-tile
CHW = N // NCH              # chunk width (2048)
GRP = 2048                  # stage-1 top-8 group width
A = 16.0                    # lhs scale

_CACHE = {}


def _build_program(f32r_dram):
    import concourse.mybir as mybir
    import concourse.tile as tile
    from concourse import bacc

    f32 = mybir.dt.float32
    f32r = mybir.dt.float32r
    f16 = mybir.dt.float16
    AX = mybir.AxisListType
    OP = mybir.AluOpType
    AF = mybir.ActivationFunctionType

    nc = bacc.Bacc(
        "TRN2", target_bir_lowering=False, debug=False, num_devices=NCORES
    )

    xdt = f32r if f32r_dram else f32
    xT_d = nc.dram_tensor("xT", [D, N], xdt, kind="ExternalInput").ap()
    lhsT_d = nc.dram_tensor("lhsT", [D, ROWS], xdt, kind="ExternalInput").ap()
    sqaug_d = nc.dram_tensor("sqaug", [128, NRT], f32, kind="ExternalInput").ap()
    zoff_d = nc.dram_tensor("zoff", [ROWS, N], f16, kind="ExternalInput").ap()
    outd = nc.dram_tensor("out", [128, NRT * 3], f32, kind="ExternalOutput").ap()

    TWO23 = float(2.0**23)

    with tile.TileContext(nc) as tc:
        with (
            tc.tile_pool(name="persist", bufs=1) as pp,
            tc.tile_pool(name="zoffs", bufs=4) as zp,
            tc.tile_pool(name="rs", bufs=4) as rp,
            tc.tile_pool(name="zs", bufs=4) as sp,
            tc.tile_pool(name="cand", bufs=2) as cp,
            tc.tile_pool(name="small", bufs=2) as smp,
            tc.tile_pool(name="psum", bufs=2, space="PSUM") as psp,
        ):
            sqaug_sb = pp.tile([128, NRT], f32, tag="sqaug")
            lhsT_sb = pp.tile([D, ROWS], f32r, tag="lhsT")
            xT_sb = pp.tile([D, N], f32r, tag="xT")
            # xT arrives in graduated slices so the first matmul waits only
            # on 256KB; the first row-tile's zoff DMA is dispatched right
            # after (the sync queue issues descriptors serially at ~0.6us
            # each, so dispatch order is latency-critical).
            XSL = [(0, 512), (512, 1024), (1024, 2048), (2048, 4096), (4096, 8192)]
            zo_tiles = {}
            if f32r_dram:
                nc.sync.dma_start(out=lhsT_sb[:], in_=lhsT_d)
                s0, s1 = XSL[0]
                nc.sync.dma_start(out=xT_sb[:, s0:s1], in_=xT_d[:, s0:s1])
                zo_tiles[0] = zp.tile([128, N], f16, tag="zo", name="zo0")
                nc.sync.dma_start(out=zo_tiles[0][:], in_=zoff_d[0:128, :])
                for s0, s1 in XSL[1:]:
                    nc.sync.dma_start(out=xT_sb[:, s0:s1], in_=xT_d[:, s0:s1])
                nc.sync.dma_start(out=sqaug_sb[:], in_=sqaug_d)
            else:
                # f32r matmul operands must come from a rounding engine op,
                # not a DMA: stage as f32, convert with DVE copies (sliced
                # so the first matmul only waits on the first slice).
                lhsT_st = pp.tile([D, ROWS], f32, tag="lhsT_st")
                nc.sync.dma_start(out=lhsT_st[:], in_=lhsT_d)
                nc.vector.tensor_copy(lhsT_sb[:], lhsT_st[:])
                xT_st = pp.tile([D, N], f32, tag="xT_st")
                for s0, s1 in XSL:
                    nc.sync.dma_start(out=xT_st[:, s0:s1], in_=xT_d[:, s0:s1])
                    nc.vector.tensor_copy(xT_sb[:, s0:s1], xT_st[:, s0:s1])
                zo_tiles[0] = zp.tile([128, N], f16, tag="zo", name="zo0")
                nc.sync.dma_start(out=zo_tiles[0][:], in_=zoff_d[0:128, :])
                nc.sync.dma_start(out=sqaug_sb[:], in_=sqaug_d)

            out_sb = pp.tile([128, NRT * 3], f32, tag="out")

            ngrp = N // GRP  # stage-1 groups per row (4)
            for rt in range(NRT):
                ce = cp.tile([128, ngrp * 8], f16, tag="ce")
                if rt not in zo_tiles:
                    zo_tiles[rt] = zp.tile([128, N], f16, tag="zo", name=f"zo{rt}")
                    nc.sync.dma_start(
                        out=zo_tiles[rt][:],
                        in_=zoff_d[rt * 128 : (rt + 1) * 128, :],
                    )
                zo = zo_tiles.pop(rt)
                for ch in range(NCH):
                    ps = psp.tile([128, CHW], f32, tag="ps")
                    for h in range(CHW // 512):
                        c0 = ch * CHW + h * 512
                        nc.tensor.matmul(
                            ps[:, h * 512 : (h + 1) * 512],
                            lhsT_sb[:, rt * 128 : (rt + 1) * 128],
                            xT_sb[:, c0 : c0 + 512],
                            start=True,
                            stop=True,
                        )
                    # scalar engine: r1 = fp16(psum + 3072) -- RNE at ulp=2
                    # rounds A*x.x to even integers
                    r1 = rp.tile([128, CHW], f16, tag="r1")
                    nc.scalar.activation(
                        r1[:], ps[:], AF.Copy, bias=3072.0, scale=1.0
                    )
                    # z = (r1 - 1872) + zoff   (all fp16)
                    zt = sp.tile([128, CHW], f16, tag="zt")
                    nc.vector.scalar_tensor_tensor(
                        zt[:], r1[:], -1872.0,
                        zo[:, ch * CHW : (ch + 1) * CHW],
                        op0=OP.add, op1=OP.add,
                    )
                    nc.vector.max(ce[:, ch * 8 : ch * 8 + 8], zt[:])

                # stage 2: exact top-16 from the 32 candidates, then the
                # per-rt postlude inline so there is no serial tail.
                m16 = smp.tile([128, 16], f16, tag="m16")
                ce2 = smp.tile([128, ngrp * 8], f16, tag="ce2")
                nc.vector.max(m16[:, 0:8], ce[:])
                nc.vector.match_replace(
                    out=ce2[:], in_to_replace=m16[:, 0:8], in_values=ce[:],
                    imm_value=-30000.0,
                )
                nc.vector.max(m16[:, 8:16], ce2[:])

                # a = z/2-0.25 (exact); k = RNE(a+2^23)-2^23; bit = z-2k
                sf = smp.tile([128, 16], f32, tag="sf")
                nc.vector.tensor_copy(sf[:], m16[:])
                a_t = smp.tile([128, 16], f32, tag="a")
                nc.vector.tensor_scalar(
                    a_t[:], sf[:], 0.5, 0.25, op0=OP.mult, op1=OP.subtract
                )
                k_t = smp.tile([128, 16], f32, tag="k")
                nc.vector.tensor_scalar(
                    k_t[:], a_t[:], TWO23, TWO23, op0=OP.add, op1=OP.subtract
                )
                bit_t = smp.tile([128, 16], f32, tag="bit")
                nc.vector.scalar_tensor_tensor(
                    bit_t[:], k_t[:], -2.0, sf[:], op0=OP.mult, op1=OP.add
                )
                # d2 = sqaug - k/(A/4); dist = sqrt(d2); e = exp(-dist)
                dist_t = smp.tile([128, 16], f32, tag="dist")
                nc.scalar.activation(
                    dist_t[:], k_t[:], AF.Sqrt,
                    bias=sqaug_sb[:, rt : rt + 1], scale=-4.0 / A,
                )
                e_t = smp.tile([128, 16], f32, tag="e")
                nc.scalar.activation(
                    e_t[:], dist_t[:], AF.Exp, scale=-1.0,
                    accum_out=out_sb[:, rt * 3 : rt * 3 + 1],
                )
                junk = smp.tile([128, 16], f32, tag="junk")
                nc.vector.scalar_tensor_tensor(
                    junk[:], e_t[:], 1.0, bit_t[:],
                    op0=OP.mult, op1=OP.mult,
                    accum_out=out_sb[:, rt * 3 + 1 : rt * 3 + 2],
                )
                nc.vector.tensor_reduce(
                    out_sb[:, rt * 3 + 2 : rt * 3 + 3], bit_t[:],
                    axis=AX.X, op=OP.add,
                )

            nc.sync.dma_start(out=outd, in_=out_sb[:])

    nc.compile()
    return nc


def _host_inputs(X, T):
    """Per-core input dicts. Core c's columns are rotated by c*ROWS."""
    X = X.astype(np.float32)
    sq = np.sum(X * X, axis=1)  # f32 [N]
    w2 = 2.0 * np.minimum(np.round((A / 4) * sq.astype(np.float64)), 1023.0)
    rng = np.random.default_rng(12345)
    sgn = rng.integers(0, 2, size=N).astype(np.float64) * 2.0 - 1.0
    Tl = T.astype(np.int64)
    in_maps = []
    for c in range(NCORES):
        rot = np.roll(np.arange(N), -c * ROWS)
        rows = slice(c * ROWS, (c + 1) * ROWS)
        sm = (Tl[rows, None] == Tl[rot][None, :]).astype(np.float64)
        zoff = (sgn[rot][None, :] * sm - w2[rot][None, :]).astype(np.float16)
        idx = np.arange(ROWS)
        zoff[idx, idx] = np.float16(-8000.0)
        in_maps.append(
            {
                "xT": np.ascontiguousarray(X[rot].T),
                "lhsT": np.ascontiguousarray((A * X[rows]).T),
                "sqaug": np.ascontiguousarray(
                    (sq[rows] + np.float32(150.0)).reshape(NRT, 128).T
                ),
                "zoff": zoff,
            }
        )
    return in_maps


def _postlude(X, T, s_tot, s_pos, cnt_pos):
    """Host finish: fallback pairs, valid mask, final 4 scalars."""
    n = N
    Xf = X.astype(np.float64)
    sq = np.sum(X.astype(np.float32) * X.astype(np.float32), axis=1).astype(
        np.float64
    )

    cnt_pos = np.round(cnt_pos).astype(np.int64)
    count_neg = 16 - cnt_pos
    neg_logit = s_tot.astype(np.float64) - s_pos.astype(np.float64)
    neg_logit = np.maximum(neg_logit, 0.0)

    # first same-label off-diagonal index per row (order of original columns)
    first_pos = np.zeros(n, dtype=np.int64)
    order = np.argsort(T, kind="stable")
    from collections import defaultdict

    by_label = defaultdict(list)
    for idx in order:
        by_label[int(T[idx])].append(int(idx))
    for i in range(n):
        lst = by_label[int(T[i])]
        if len(lst) >= 2:
            first_pos[i] = lst[1] if lst[0] == i else lst[0]
        else:
            first_pos[i] = 0  # no positives; row is invalid anyway

    j = first_pos
    d2 = sq + sq[j] - 2.0 * np.einsum("ij,ij->i", Xf, Xf[j])
    fb_dist = np.sqrt(np.maximum(d2, 1e-12))
    fallback = np.exp(-fb_dist)

    counts = np.bincount(T.astype(np.int64), minlength=128)
    same_cnt = counts[T.astype(np.int64)] - 1
    valid = (same_cnt > 0) & ((n - 1 - same_cnt) > 0)

    pos_eff = np.where(cnt_pos == 0, fallback, s_pos.astype(np.float64))
    loss_i = -np.log(pos_eff / (pos_eff + neg_logit))
    loss = np.sum(np.where(valid, loss_i, 0.0)) / n

    count_pos_acc = np.where(cnt_pos == 0, 1, cnt_pos)
    accuracy = np.sum((valid & (count_pos_acc > count_neg)).astype(np.float64)) / n
    tp = np.sum(np.where(valid, cnt_pos, 0)) / n
    tn = np.sum(np.where(valid, count_neg, 0)) / n
    return (
        np.float32(loss),
        np.float32(accuracy),
        np.float32(tp),
        np.float32(tn),
    )


def _get_nc():
    if "nc" in _CACHE:
        return _CACHE["nc"]
    _CACHE["nc"] = _build_program(f32r_dram=_CACHE.get("f32r_dram", True))
    return _CACHE["nc"]


def kernel(inputs, targets):
    from concourse.bass_utils import run_bass_kernel_spmd

    X = np.asarray(inputs, dtype=np.float32)
    T = np.asarray(targets).astype(np.int64)

    in_maps = _host_inputs(X, T)
    try:
        nc = _get_nc()
        res = run_bass_kernel_spmd(nc, in_maps, core_ids=list(range(NCORES)))
    except Exception:
        if _CACHE.get("f32r_dram", True) is False:
            raise
        # f32r-typed DRAM inputs rejected by the BIR verifier on this
        # toolchain: rebuild with f32 staging + DVE conversion.
        _CACHE.pop("nc", None)
        _CACHE["f32r_dram"] = False
        nc = _get_nc()
        res = run_bass_kernel_spmd(nc, in_maps, core_ids=list(range(NCORES)))

    s_tot = np.zeros(N, dtype=np.float64)
    s_pos = np.zeros(N, dtype=np.float64)
    cnt_pos = np.zeros(N, dtype=np.float64)
    for c in range(NCORES):
        out = res.results[c]["out"]  # [128, NRT*3]
        for rt in range(NRT):
            g = slice(c * ROWS + rt * 128, c * ROWS + (rt + 1) * 128)
            s_tot[g] = out[:, rt * 3]
            s_pos[g] = out[:, rt * 3 + 1]
            cnt_pos[g] = out[:, rt * 3 + 2]

    return _postlude(X, T, s_tot, s_pos, cnt_pos)


# revision 19
# speedup vs baseline: 6.2678x; 1.0245x over previous
"""KNN-softmax loss kernel for Trainium2, SPMD over 8 NeuronCores.

Problem: N=8192 points, D=128, 128 classes, K=16, alpha=1.
The reference's per-row threshold (17th smallest off-diagonal distance)
means the selected set is exactly the 16 nearest off-diagonal columns per
row; the label mask only matters on those 16 elements.

Sharding: rows data-parallel across 8 cores (1024 rows each); every core
holds all N column embeddings.

fp16 single-scan z-encoding:
  PE computes psum = A*(x_r . x_c), A=16, baked into the row block.
  The scalar engine rounds: r1 = fp16(psum + 3072) -- RNE into [2048,4096)
  where the fp16 ulp is 2, so r1 = 3072 + 2*round(8*x.x) exactly (even).
  One DVE scalar_tensor_tensor (all fp16) forms
      z = (r1 - 1872) + zoff[r, c]
  with the host table zoff = sgn_c*samelabel - 2*round(4*||x_c||^2)
  (fp16-exact integers; sgn_c a fixed per-column +-1 dither that cancels
  the tie-break bias; diagonal entries get -8000). So
      z = 1200 + 2*(round(8 x.x) - w_c) + sgn_c*samelabel
  exactly: ordering by z == ordering by negated squared distance
  (quantized to 0.125 d2 units), with the label-match bit in the LSB.
  A single hierarchical fp16 max8 scan (top-8 per 1024-group, then
  top-16 of 64 candidates) yields the 16 nearest columns together with
  their label bits. A tiny postlude recovers k via an RNE trick,
  bit = z - 2k, d2 = ||x_r||^2 + 150 - k/4, and sqrt/exp on [128,16]
  tiles only.

The O(N) host postlude (fallback pairs, valid mask, final scalars) is
unchanged from the baseline.
"""

import numpy as np

N, D, NCORES = 8192, 128, 8
ROWS = N // NCORES          # rows per core
NRT = ROWS // 128           # row-tiles per core
NCH = 4                     # column chunks per row-tile
CHW = N // NCH              # chunk width (2048)
GRP = 2048                  # stage-1 top-8 group width
A = 16.0                    # lhs scale

_CACHE = {}


def _build_program(f32r_dram):
    import concourse.mybir as mybir
    import concourse.tile as tile
    from concourse import bacc

    f32 = mybir.dt.float32
    f32r = mybir.dt.float32r
    f16 = mybir.dt.float16
    AX = mybir.AxisListType
    OP = mybir.AluOpType
    AF = mybir.ActivationFunctionType

    nc = bacc.Bacc(
        "TRN2", target_bir_lowering=False, debug=False, num_devices=NCORES
    )

    xdt = f32r if f32r_dram else f32
    xT_d = nc.dram_tensor("xT", [D, N], xdt, kind="ExternalInput").ap()
    lhsT_d = nc.dram_tensor("lhsT", [D, ROWS], xdt, kind="ExternalInput").ap()
    sqaug_d = nc.dram_tensor("sqaug", [128, NRT], f32, kind="ExternalInput").ap()
    zoff_d = nc.dram_tensor("zoff", [ROWS, N], f16, kind="ExternalInput").ap()
    outd = nc.dram_tensor("out", [128, NRT * 3], f32, kind="ExternalOutput").ap()

    TWO23 = float(2.0**23)

    with tile.TileContext(nc) as tc:
        with (
            tc.tile_pool(name="persist", bufs=1) as pp,
            tc.tile_pool(name="zoffs", bufs=4) as zp,
            tc.tile_pool(name="rs", bufs=4) as rp,
            tc.tile_pool(name="zs", bufs=4) as sp,
            tc.tile_pool(name="cand", bufs=2) as cp,
            tc.tile_pool(name="small", bufs=2) as smp,
            tc.tile_pool(name="psum", bufs=2, space="PSUM") as psp,
        ):
            sqaug_sb = pp.tile([128, NRT], f32, tag="sqaug")
            lhsT_sb = pp.tile([D, ROWS], f32r, tag="lhsT")
            xT_sb = pp.tile([D, N], f32r, tag="xT")
            # xT arrives in graduated slices so the first matmul waits only
            # on 256KB; the first row-tile's zoff DMA is dispatched right
            # after (the sync queue issues descriptors serially at ~0.6us
            # each, so dispatch order is latency-critical).
            XSL = [(0, 512), (512, 1024), (1024, 2048), (2048, 4096), (4096, 8192)]
            zo_tiles = {}
            if f32r_dram:
                nc.sync.dma_start(out=lhsT_sb[:], in_=lhsT_d)
                s0, s1 = XSL[0]
                nc.sync.dma_start(out=xT_sb[:, s0:s1], in_=xT_d[:, s0:s1])
                # rt0's zoff lands chunk-by-chunk, interleaved with the xT
                # slices, so STT(0,0) waits on 512KB, not 2MB.
                zo_tiles[0] = zp.tile([128, N], f16, tag="zo", name="zo0")
                for j, (s0, s1) in enumerate(XSL[1:]):
                    nc.sync.dma_start(
                        out=zo_tiles[0][:, j * CHW : (j + 1) * CHW],
                        in_=zoff_d[0:128, j * CHW : (j + 1) * CHW],
                    )
                    nc.sync.dma_start(out=xT_sb[:, s0:s1], in_=xT_d[:, s0:s1])
                nc.sync.dma_start(out=sqaug_sb[:], in_=sqaug_d)
            else:
                # f32r matmul operands must come from a rounding engine op,
                # not a DMA: stage as f32, convert with DVE copies (sliced
                # so the first matmul only waits on the first slice).
                lhsT_st = pp.tile([D, ROWS], f32, tag="lhsT_st")
                nc.sync.dma_start(out=lhsT_st[:], in_=lhsT_d)
                nc.vector.tensor_copy(lhsT_sb[:], lhsT_st[:])
                xT_st = pp.tile([D, N], f32, tag="xT_st")
                for s0, s1 in XSL:
                    nc.sync.dma_start(out=xT_st[:, s0:s1], in_=xT_d[:, s0:s1])
                    nc.vector.tensor_copy(xT_sb[:, s0:s1], xT_st[:, s0:s1])
                zo_tiles[0] = zp.tile([128, N], f16, tag="zo", name="zo0")
                nc.sync.dma_start(out=zo_tiles[0][:], in_=zoff_d[0:128, :])
                nc.sync.dma_start(out=sqaug_sb[:], in_=sqaug_d)

            out_sb = pp.tile([128, NRT * 3], f32, tag="out")
            stash = pp.tile([128, NRT * 16], f16, tag="stash")

            def dev_postlude(r0, r1):
                """Recover (k, bit) from stashed z values for rts [r0, r1),
                then d2 = sqaug - k/(A/4), e = exp(-sqrt(d2)), and the three
                per-rt row stats."""
                w = (r1 - r0) * 16
                sl = slice(r0 * 16, r1 * 16)
                sf = smp.tile([128, w], f32, tag="sf", name=f"sf{r0}")
                nc.vector.tensor_copy(sf[:], stash[:, sl])
                a_t = smp.tile([128, w], f32, tag="a", name=f"a{r0}")
                nc.vector.tensor_scalar(
                    a_t[:], sf[:], 0.5, 0.25, op0=OP.mult, op1=OP.subtract
                )
                k_t = smp.tile([128, w], f32, tag="k", name=f"k{r0}")
                nc.vector.tensor_scalar(
                    k_t[:], a_t[:], TWO23, TWO23, op0=OP.add, op1=OP.subtract
                )
                bit_t = smp.tile([128, w], f32, tag="bit", name=f"bit{r0}")
                nc.vector.scalar_tensor_tensor(
                    bit_t[:], k_t[:], -2.0, sf[:], op0=OP.mult, op1=OP.add
                )
                dist_t = smp.tile([128, w], f32, tag="dist", name=f"dist{r0}")
                for rt in range(r0, r1):
                    v = slice((rt - r0) * 16, (rt - r0 + 1) * 16)
                    nc.scalar.activation(
                        dist_t[:, v], k_t[:, v], AF.Sqrt,
                        bias=sqaug_sb[:, rt : rt + 1], scale=-4.0 / A,
                    )
                e_t = smp.tile([128, w], f32, tag="e", name=f"e{r0}")
                for rt in range(r0, r1):
                    v = slice((rt - r0) * 16, (rt - r0 + 1) * 16)
                    nc.scalar.activation(
                        e_t[:, v], dist_t[:, v], AF.Exp, scale=-1.0,
                        accum_out=out_sb[:, rt * 3 : rt * 3 + 1],
                    )
                junk = smp.tile([128, w], f32, tag="junk", name=f"junk{r0}")
                for rt in range(r0, r1):
                    v = slice((rt - r0) * 16, (rt - r0 + 1) * 16)
                    nc.vector.scalar_tensor_tensor(
                        junk[:, v], e_t[:, v], 1.0, bit_t[:, v],
                        op0=OP.mult, op1=OP.mult,
                        accum_out=out_sb[:, rt * 3 + 1 : rt * 3 + 2],
                    )
                    nc.vector.tensor_reduce(
                        out_sb[:, rt * 3 + 2 : rt * 3 + 3], bit_t[:, v],
                        axis=AX.X, op=OP.add,
                    )

            ngrp = N // GRP  # stage-1 groups per row (4)
            for rt in range(NRT):
                ce = cp.tile([128, ngrp * 8], f16, tag="ce")
                if rt not in zo_tiles:
                    zo_tiles[rt] = zp.tile([128, N], f16, tag="zo", name=f"zo{rt}")
                    nc.sync.dma_start(
                        out=zo_tiles[rt][:],
                        in_=zoff_d[rt * 128 : (rt + 1) * 128, :],
                    )
                zo = zo_tiles.pop(rt)
                for ch in range(NCH):
                    ps = psp.tile([128, CHW], f32, tag="ps")
                    for h in range(CHW // 512):
                        c0 = ch * CHW + h * 512
                        nc.tensor.matmul(
                            ps[:, h * 512 : (h + 1) * 512],
                            lhsT_sb[:, rt * 128 : (rt + 1) * 128],
                            xT_sb[:, c0 : c0 + 512],
                            start=True,
                            stop=True,
                        )
                    # scalar engine: r1 = fp16(psum + 3072) -- RNE at ulp=2
                    # rounds A*x.x to even integers
                    r1 = rp.tile([128, CHW], f16, tag="r1")
                    nc.scalar.activation(
                        r1[:], ps[:], AF.Copy, bias=3072.0, scale=1.0
                    )
                    # z = (r1 - 1872) + zoff   (all fp16)
                    zt = sp.tile([128, CHW], f16, tag="zt")
                    nc.vector.scalar_tensor_tensor(
                        zt[:], r1[:], -1872.0,
                        zo[:, ch * CHW : (ch + 1) * CHW],
                        op0=OP.add, op1=OP.add,
                    )
                    nc.vector.max(ce[:, ch * 8 : ch * 8 + 8], zt[:])

                # stage 2: exact top-16 from the 32 candidates, into stash
                o = rt * 16
                ce2 = smp.tile([128, ngrp * 8], f16, tag="ce2")
                nc.vector.max(stash[:, o : o + 8], ce[:])
                nc.vector.match_replace(
                    out=ce2[:], in_to_replace=stash[:, o : o + 8], in_values=ce[:],
                    imm_value=-30000.0,
                )
                nc.vector.max(stash[:, o + 8 : o + 16], ce2[:])

                if rt == NRT - 2:
                    # batched postlude for rts 0..NRT-2 while rt NRT-1
                    # still computes; only the last rt runs in the tail.
                    dev_postlude(0, NRT - 1)
            dev_postlude(NRT - 1, NRT)
            nc.sync.dma_start(out=outd, in_=out_sb[:])

    nc.compile()
    return nc


def _host_inputs(X, T):
    """Per-core input dicts. Core c's columns are rotated by c*ROWS."""
    X = X.astype(np.float32)
    sq = np.sum(X * X, axis=1)  # f32 [N]
    w2 = 2.0 * np.minimum(np.round((A / 4) * sq.astype(np.float64)), 1023.0)
    rng = np.random.default_rng(12345)
    sgn = rng.integers(0, 2, size=N).astype(np.float64) * 2.0 - 1.0
    Tl = T.astype(np.int64)
    in_maps = []
    for c in range(NCORES):
        rot = np.roll(np.arange(N), -c * ROWS)
        rows = slice(c * ROWS, (c + 1) * ROWS)
        sm = (Tl[rows, None] == Tl[rot][None, :]).astype(np.float64)
        zoff = (sgn[rot][None, :] * sm - w2[rot][None, :]).astype(np.float16)
        idx = np.arange(ROWS)
        zoff[idx, idx] = np.float16(-8000.0)
        in_maps.append(
            {
                "xT": np.ascontiguousarray(X[rot].T),
                "lhsT": np.ascontiguousarray((A * X[rows]).T),
                "sqaug": np.ascontiguousarray(
                    (sq[rows] + np.float32(150.0)).reshape(NRT, 128).T
                ),
                "zoff": zoff,
            }
        )
    return in_maps


def _postlude(X, T, s_tot, s_pos, cnt_pos):
    """Host finish: fallback pairs, valid mask, final 4 scalars."""
    n = N
    Xf = X.astype(np.float64)
    sq = np.sum(X.astype(np.float32) * X.astype(np.float32), axis=1).astype(
        np.float64
    )

    cnt_pos = np.round(cnt_pos).astype(np.int64)
    count_neg = 16 - cnt_pos
    neg_logit = s_tot.astype(np.float64) - s_pos.astype(np.float64)
    neg_logit = np.maximum(neg_logit, 0.0)

    # first same-label off-diagonal index per row (order of original columns)
    first_pos = np.zeros(n, dtype=np.int64)
    order = np.argsort(T, kind="stable")
    from collections import defaultdict

    by_label = defaultdict(list)
    for idx in order:
        by_label[int(T[idx])].append(int(idx))
    for i in range(n):
        lst = by_label[int(T[i])]
        if len(lst) >= 2:
            first_pos[i] = lst[1] if lst[0] == i else lst[0]
        else:
            first_pos[i] = 0  # no positives; row is invalid anyway

    j = first_pos
    d2 = sq + sq[j] - 2.0 * np.einsum("ij,ij->i", Xf, Xf[j])
    fb_dist = np.sqrt(np.maximum(d2, 1e-12))
    fallback = np.exp(-fb_dist)

    counts = np.bincount(T.astype(np.int64), minlength=128)
    same_cnt = counts[T.astype(np.int64)] - 1
    valid = (same_cnt > 0) & ((n - 1 - same_cnt) > 0)

    pos_eff = np.where(cnt_pos == 0, fallback, s_pos.astype(np.float64))
    loss_i = -np.log(pos_eff / (pos_eff + neg_logit))
    loss = np.sum(np.where(valid, loss_i, 0.0)) / n

    count_pos_acc = np.where(cnt_pos == 0, 1, cnt_pos)
    accuracy = np.sum((valid & (count_pos_acc > count_neg)).astype(np.float64)) / n
    tp = np.sum(np.where(valid, cnt_pos, 0)) / n
    tn = np.sum(np.where(valid, count_neg, 0)) / n
    return (
        np.float32(loss),
        np.float32(accuracy),
        np.float32(tp),
        np.float32(tn),
    )


def _get_nc():
    if "nc" in _CACHE:
        return _CACHE["nc"]
    _CACHE["nc"] = _build_program(f32r_dram=_CACHE.get("f32r_dram", True))
    return _CACHE["nc"]


def kernel(inputs, targets):
    from concourse.bass_utils import run_bass_kernel_spmd

    X = np.asarray(inputs, dtype=np.float32)
    T = np.asarray(targets).astype(np.int64)

    in_maps = _host_inputs(X, T)
    try:
        nc = _get_nc()
        res = run_bass_kernel_spmd(nc, in_maps, core_ids=list(range(NCORES)))
    except Exception:
        if _CACHE.get("f32r_dram", True) is False:
            raise
        # f32r-typed DRAM inputs rejected by the BIR verifier on this
        # toolchain: rebuild with f32 staging + DVE conversion.
        _CACHE.pop("nc", None)
        _CACHE["f32r_dram"] = False
        nc = _get_nc()
        res = run_bass_kernel_spmd(nc, in_maps, core_ids=list(range(NCORES)))

    s_tot = np.zeros(N, dtype=np.float64)
    s_pos = np.zeros(N, dtype=np.float64)
    cnt_pos = np.zeros(N, dtype=np.float64)
    for c in range(NCORES):
        out = res.results[c]["out"]  # [128, NRT*3]
        for rt in range(NRT):
            g = slice(c * ROWS + rt * 128, c * ROWS + (rt + 1) * 128)
            s_tot[g] = out[:, rt * 3]
            s_pos[g] = out[:, rt * 3 + 1]
            cnt_pos[g] = out[:, rt * 3 + 2]

    return _postlude(X, T, s_tot, s_pos, cnt_pos)
